# revision 27
# baseline (speedup 1.0000x reference)
"""Trainium2 Bass kernel for a multi-head self-attention block.

Reference computation (B=4, N=2048, D=256, H=8, dh=32, DFF=512):
    x_ln = LN0(x); Q = x_ln@Wq.T+bq; K = y@Wk.T+bk; V = y@Wv.T+bv
    per head: A = softmax(Qh Kh^T / 16); O = concat_h(Qh + A Vh)
    out = O + (gelu(LN1(O)@W1.T+b1) @ W2.T + b2)

Sharding: 8 cores = 4 batches x 2 halves of the query sequence. Each core
gets its x half-shard and the full y for its batch; no collectives.

Layout: feature-on-partition ("transposed") everywhere. The 256 feature
dims of Q/O are spread over a 512-slot space [128 partitions, 4 ktiles]:
head h lives at partition strip 64*(h%2)..+32, ktile o=h//2. LN folds,
head permutation, and the V-bias fold (bv moves into bq since sum(A)=1)
are host-side weight prep. No max-subtraction in softmax (|s/16|<~1.5).

The attention bmm-softmax-bmm runs in fp8e4m3 with DoubleRow matmuls
(2 contraction rows/cycle): scores contract dh=32 as [16 partitions x 2
subtiles] per head (Q/K relayed out to 16-partition strips by on-chip
DMA), and AV contracts 256 keys (2 key tiles) per matmul with V stored
as [128 tokens, ktpair, 2, head*32]. Softmax denominators come from a
dedicated DoubleRow matmul with a [1|0...] fp8 stationary, landing on
PSUM rows 32/96 of the AV accumulator. Projections/FFN stay bf16; PSUM
accumulation is fp32 throughout.

The exp is split act/DVE by key-tile parity: even tiles get the exact
Act-engine Exp (fp8 out), odd tiles a Schraudolph exp on the DVE (one
tensor_scalar building fp8e4m3 bit patterns in int8; its +-4% equi-
ripple error is common-mode across the softmax and mostly cancels).
Softmax and LN reciprocals run on GPSIMD as a Mitchell bit-trick seed +
one Newton step (standard convert/mult ops; GPSIMD cannot touch PSUM,
so Act first copies the denominator row to SBUF). The seed produces
-1/d; the broadcast matmul's stationary is -1 to flip it back.

Engines have in-order queues, so issue order is the main scheduling
tool: K/V projections are issued inside the LN0 scalar chain, each
attention iteration's epilogue is deferred into the next iteration's
key loop, and LN1/FFN/output-DMA are chunked per 512 tokens and
interleaved into the tail attention iterations.
"""

import contextlib

import numpy as np

B, N, D = 4, 2048, 256
H, DH, DFF = 8, 32, 512
P = 128
NTOK = N // 2            # query tokens per core
NQT = NTOK // 512        # q tiles of 512
NKT = N // P             # key tiles of 128
SCALE = 1.0 / 16.0
EPS = 1e-5
DSLOT = 512              # padded feature-slot space for Q/O

# Schraudolph exp constants for bf16 bit patterns in int16:
#   bits = round(s * SCALE*128*log2(e) + (127*128 - c8))
LOG2E = 1.4426950408889634
SCH_A = SCALE * 128.0 * LOG2E
SCH_B = 127.0 * 128.0 - 366392.5 / 65536.0
# Mitchell reciprocal seed: bits(y0) = C2 - bits(d); one Newton step
# (u-2)*y0 then gives -1/d to ~0.3%.
RCP_C2 = 2129834424.0

_NC_CACHE = {}


def _slot(h, i):
    return (h // 2) * P + 64 * (h % 2) + i


def _build_nc():
    import concourse.mybir as mybir
    import concourse.tile as tile
    from concourse import bacc

    f32 = mybir.dt.float32
    bf16 = mybir.dt.bfloat16
    fp8 = mybir.dt.float8e4
    i16 = mybir.dt.int16
    i32 = mybir.dt.int32
    AF = mybir.ActivationFunctionType
    ALU = mybir.AluOpType
    DR = mybir.MatmulPerfMode.DoubleRow

    nc = bacc.Bacc("TRN2", target_bir_lowering=False, debug=False)

    xt_d = nc.dram_tensor("xt", [D, NTOK], bf16, kind="ExternalInput")
    yt_d = nc.dram_tensor("yt", [D, N], bf16, kind="ExternalInput")
    wq_d = nc.dram_tensor("wq", [D, DSLOT], bf16, kind="ExternalInput")
    bq_d = nc.dram_tensor("bq", [DSLOT], f32, kind="ExternalInput")
    wk_d = nc.dram_tensor("wk", [D, DSLOT], bf16, kind="ExternalInput")
    bk_d = nc.dram_tensor("bk", [DSLOT], f32, kind="ExternalInput")
    wv_d = nc.dram_tensor("wv", [D, H * 33], bf16, kind="ExternalInput")
    w1_d = nc.dram_tensor("w1", [DSLOT, DFF], bf16, kind="ExternalInput")
    b1_d = nc.dram_tensor("b1", [DFF], f32, kind="ExternalInput")
    w2_d = nc.dram_tensor("w2", [DFF + 1, DSLOT], bf16, kind="ExternalInput")
    out_d = nc.dram_tensor("out_t", [D, NTOK], f32, kind="ExternalOutput")

    with tile.TileContext(nc) as tc, contextlib.ExitStack() as ctx:
        const = ctx.enter_context(tc.tile_pool(name="const", bufs=1))
        big = ctx.enter_context(tc.tile_pool(name="big", bufs=1))
        scratch = ctx.enter_context(tc.tile_pool(name="scratch", bufs=1))
        apool = ctx.enter_context(tc.tile_pool(name="apool", bufs=4))
        # PSUM: scores 2x[128,1024]=4 banks, av 2, bc 1, proj 1.
        scores_pool = ctx.enter_context(
            tc.tile_pool(name="scoresp", bufs=2, space="PSUM"))
        av_pool = ctx.enter_context(tc.tile_pool(name="avp", bufs=2, space="PSUM"))
        bc_pool = ctx.enter_context(tc.tile_pool(name="bcp", bufs=1, space="PSUM"))
        proj_pool = ctx.enter_context(tc.tile_pool(name="projp", bufs=1, space="PSUM"))

        # ---- constants / inputs (K-proj inputs first) ---------------------
        yt_s = big.tile([P, 2, N], bf16)
        nc.sync.dma_start(yt_s[:], yt_d.rearrange("(o p) t -> p o t", p=P))
        wk_s = const.tile([P, 2, DSLOT], bf16)
        nc.sync.dma_start(wk_s[:], wk_d.rearrange("(o p) m -> p o m", p=P))
        xt_s = big.tile([P, 2, NTOK], bf16)
        nc.sync.dma_start(xt_s[:], xt_d.rearrange("(o p) t -> p o t", p=P))
        wv_s = const.tile([P, 2, H * 33], bf16)
        nc.sync.dma_start(wv_s[:], wv_d.rearrange("(o p) m -> p o m", p=P))
        wq_s = const.tile([P, 2, DSLOT], bf16)
        nc.sync.dma_start(wq_s[:], wq_d.rearrange("(o p) m -> p o m", p=P))
        w1_s = const.tile([P, 4, DFF], bf16)
        nc.sync.dma_start(w1_s[:], w1_d.rearrange("(o p) m -> p o m", p=P))
        w2_s = const.tile([P, 5, DSLOT], bf16)
        nc.sync.dma_start(w2_s[:, 0:4, :],
                          w2_d[0:DFF, :].rearrange("(o p) m -> p o m", p=P))
        nc.sync.dma_start(w2_s[0:1, 4, :], w2_d[DFF:, :])
        bq_s = const.tile([P, 4], f32)
        nc.sync.dma_start(bq_s[:], bq_d.rearrange("(m p) -> p m", p=P))
        bk_s = const.tile([P, 4], f32)
        nc.sync.dma_start(bk_s[:], bk_d.rearrange("(m p) -> p m", p=P))
        b1_s = const.tile([P, 4], f32)
        nc.sync.dma_start(b1_s[:], b1_d.rearrange("(m p) -> p m", p=P))

        ones_s = const.tile([P, 512], bf16)
        nc.vector.memset(ones_s[:], 1.0)
        negs_s = const.tile([P, 512], bf16)
        nc.vector.memset(negs_s[:], -1.0)
        eps_s = const.tile([1, 1], f32)
        nc.vector.memset(eps_s[:], EPS)


        # ---- shared LN scratch --------------------------------------------
        mean = scratch.tile([1, NTOK], f32, tag="mean")
        mean_b = scratch.tile([1, NTOK], bf16, tag="mean_b")
        rstd_b = scratch.tile([1, NTOK], bf16, tag="rstd_b")
        lt = scratch.tile([1, NTOK], f32, tag="lntmp")
        m2 = scratch.tile([1, NTOK], f32, tag="m2")
        mb_sb = scratch.tile([P, NTOK], bf16, tag="mb_sb")
        rb_sb = scratch.tile([P, NTOK], bf16, tag="rb_sb")
        # Mitchell reciprocal scratch rows
        bitsf = scratch.tile([P, 512], f32, tag="bitsf")
        y0i = scratch.tile([P, 512], i32, tag="y0i")
        uu = scratch.tile([P, 512], f32, tag="uu")

        def mitchell_recip_neg(row_ap, out_ap, rows):
            """out = -1/in via bit-trick seed + one Newton step, on GPSIMD
            (which supports only plain tensor_scalar/tensor_tensor opcodes).
            row_ap/out_ap/scratch all share the same partition rows."""
            nc.gpsimd.tensor_copy(out=bitsf[rows], in_=row_ap.bitcast(i32))
            nc.gpsimd.tensor_scalar(out=y0i[rows], in0=bitsf[rows],
                                    scalar1=-1.0, scalar2=RCP_C2,
                                    op0=ALU.mult, op1=ALU.add)
            nc.gpsimd.tensor_tensor(out=uu[rows], in0=row_ap,
                                    in1=y0i[rows].bitcast(f32), op=ALU.mult)
            nc.gpsimd.tensor_scalar(out=uu[rows], in0=uu[rows],
                                    scalar1=-2.0, scalar2=None, op0=ALU.add)
            nc.gpsimd.tensor_tensor(out=out_ap, in0=uu[rows],
                                    in1=y0i[rows].bitcast(f32), op=ALU.mult)

        def ln_sums(src, sq, no, hf):
            """Square already computed into sq; accumulate chunk sums into one
            proj_pool tile (sx at row 0, sq at row 32) -> mean / E[x^2]."""
            cs = slice(hf * 512, hf * 512 + 512)
            ps = proj_pool.tile([P, 512], f32, tag="proj", name="lnsum")
            for o in range(no):
                nc.tensor.matmul(ps[0:1, :], lhsT=ones_s[:, 0:1],
                                 rhs=src[:, o, cs],
                                 start=(o == 0), stop=(o == no - 1),
                                 tile_position=(0, 0), skip_group_check=True)
                nc.tensor.matmul(ps[32:33, :], lhsT=ones_s[:, 0:1],
                                 rhs=sq[:, o, cs],
                                 start=(o == 0), stop=(o == no - 1),
                                 tile_position=(0, 32), skip_group_check=True)
            nc.vector.tensor_scalar_mul(mean[0:1, cs], ps[0:1, :], 1.0 / D)
            nc.vector.tensor_scalar_mul(lt[0:1, cs], ps[32:33, :], 1.0 / D)

        def ln_finish(src, dst, no, hf):
            """rstd for chunk hf (GPSIMD Mitchell recip of sqrt(var+eps), as
            -1/s with the sign flipped by the negs broadcast), broadcast,
            normalize src->dst (GPSIMD)."""
            cs = slice(hf * 512, hf * 512 + 512)
            nc.vector.tensor_tensor(out=m2[0:1, cs], in0=mean[0:1, cs],
                                    in1=mean[0:1, cs], op=ALU.mult)
            nc.vector.tensor_tensor(out=lt[0:1, cs], in0=lt[0:1, cs],
                                    in1=m2[0:1, cs], op=ALU.subtract)
            nc.scalar.activation(out=lt[0:1, cs], in_=lt[0:1, cs], func=AF.Sqrt,
                                 bias=eps_s[:])
            with nc.allow_low_precision(reason="LN rstd in bf16"):
                mitchell_recip_neg(lt[0:1, cs], rstd_b[0:1, cs],
                                   (slice(0, 1), slice(0, 512)))
            nc.vector.tensor_copy(out=mean_b[0:1, cs], in_=mean[0:1, cs])
            br = scores_pool.tile([P, 1024], f32, tag="scores", name="br")
            nc.tensor.matmul(br[:, 0:512], lhsT=ones_s[0:1, 0:P],
                             rhs=mean_b[0:1, cs], start=True, stop=True)
            nc.tensor.matmul(br[:, 512:1024], lhsT=negs_s[0:1, 0:P],
                             rhs=rstd_b[0:1, cs], start=True, stop=True)
            nc.scalar.activation(out=mb_sb[:, cs], in_=br[:, 0:512], func=AF.Copy)
            nc.scalar.activation(out=rb_sb[:, cs], in_=br[:, 512:1024],
                                 func=AF.Copy)
            for o in range(no):
                nc.gpsimd.tensor_tensor(out=dst[:, o, cs], in0=src[:, o, cs],
                                        in1=mb_sb[:, cs], op=ALU.subtract)
                nc.gpsimd.tensor_tensor(out=dst[:, o, cs], in0=dst[:, o, cs],
                                        in1=rb_sb[:, cs], op=ALU.mult)

        # ---- phase A: LN0 (sums), K/V proj, LN0 finish, Q proj -------------
        xln_s = big.tile([P, 2, NTOK], bf16)
        oln_s = big.tile([P, 4, NTOK], bf16)
        sq0 = oln_s[:, 0:2, :]                 # borrow as Square scratch
        nc.scalar.activation(out=sq0[:], in_=xt_s[:], func=AF.Square)
        for hf in range(NQT):
            ln_sums(xt_s, sq0, 2, hf)

        # K proj straight to fp8 (scores are the only K consumer).
        kt8_pre = big.tile([P, 4, N], fp8)
        for mt in range(4):
            for nt in range(N // 512):
                ns_ = slice(nt * 512, nt * 512 + 512)
                ps = proj_pool.tile([P, 512], f32, tag="proj", name="ps")
                for o in range(2):
                    nc.tensor.matmul(ps[:], lhsT=wk_s[:, o, mt * P:mt * P + P],
                                     rhs=yt_s[:, o, ns_],
                                     start=(o == 0), stop=(o == 1))
                if nt % 2 == 0:
                    nc.scalar.activation(out=kt8_pre[:, mt, ns_], in_=ps[:],
                                         func=AF.Identity,
                                         bias=bk_s[:, mt:mt + 1])
                else:
                    nc.vector.tensor_scalar_add(kt8_pre[:, mt, ns_], ps[:],
                                                bk_s[:, mt:mt + 1])
        # relayout to 16-partition dh-subtile strips for DoubleRow scores
        kt8 = big.tile([P, 2, 2, N], fp8)
        for h in range(H):
            b, l = 32 * (h % 4), h // 4
            for s in range(2):
                nc.sync.dma_start(
                    kt8[b:b + 16, l, s, :],
                    kt8_pre[64 * (h % 2) + 16 * s:64 * (h % 2) + 16 * s + 16,
                            h // 2, :])
        # V in natural [token, dout] layout, 33-wide head blocks ([Vh | ones])
        v_s = big.tile([P, NKT, H * 33], bf16)
        for tt in range(NKT):
            ts_ = slice(tt * P, tt * P + P)
            ps = proj_pool.tile([P, 512], f32, tag="proj", name="ps")[:, 0:H * 33]
            for o in range(2):
                nc.tensor.matmul(ps[:], lhsT=yt_s[:, o, ts_],
                                 rhs=wv_s[:, o, :], start=(o == 0), stop=(o == 1))
            if tt % 2 == 0:
                nc.scalar.activation(out=v_s[:, tt, :], in_=ps[:], func=AF.Copy)
            else:
                nc.vector.tensor_copy(out=v_s[:, tt, :], in_=ps[:])
        for h in range(H):
            nc.gpsimd.memset(v_s[:, :, 33 * h + 32], 1.0)

        for hf in range(NQT):
            ln_finish(xt_s, xln_s, 2, hf)

        qt_s = big.tile([P, 4, NTOK], bf16)
        for mt in range(4):
            for nt in range(NQT):
                ns_ = slice(nt * 512, nt * 512 + 512)
                ps = proj_pool.tile([P, 512], f32, tag="proj", name="ps")
                for o in range(2):
                    nc.tensor.matmul(ps[:], lhsT=wq_s[:, o, mt * P:mt * P + P],
                                     rhs=xln_s[:, o, ns_],
                                     start=(o == 0), stop=(o == 1))
                nc.scalar.activation(out=qt_s[:, mt, ns_], in_=ps[:],
                                     func=AF.Identity, bias=bq_s[:, mt:mt + 1])
        qt8_pre = big.tile([P, 4, NTOK], fp8)
        nc.gpsimd.tensor_copy(out=qt8_pre[:], in_=qt_s[:])
        qt8 = big.tile([P, 2, 2, NTOK], fp8)
        for h in range(H):
            b, l = 32 * (h % 4), h // 4
            for s in range(2):
                nc.sync.dma_start(
                    qt8[b:b + 16, l, s, :],
                    qt8_pre[64 * (h % 2) + 16 * s:64 * (h % 2) + 16 * s + 16,
                            h // 2, :])

        # ---- phase B (attention) with phase C (LN1+FFN) interleaved --------
        ot_s = big.tile([P, 4, NTOK], bf16)
        nc.gpsimd.memset(ot_s[32:64, :, :], 0.0)
        nc.gpsimd.memset(ot_s[96:128, :, :], 0.0)
        h_s = big.tile([P, 4, NTOK], bf16)
        outt_s = big.tile([P, 4, NTOK], f32)
        dsb_s = scratch.tile([P, 512], f32, tag="dsb")
        rc_s = scratch.tile([P, 512], bf16, tag="rc")

        def make_epilogue(pr, qt, av):
            qs_ = slice(qt * 512, qt * 512 + 512)

            def emit():
                for jj in range(2):
                    st = 64 * jj
                    rows = (slice(st + 32, st + 33), slice(0, 512))
                    nc.scalar.activation(out=dsb_s[rows], in_=av[rows],
                                         func=AF.Copy)
                    with nc.allow_low_precision(reason="softmax recip bf16"):
                        mitchell_recip_neg(dsb_s[rows], rc_s[rows], rows)
                bc = bc_pool.tile([P, 512], f32, tag="bc", name="bc")
                for jj in range(2):
                    st = 64 * jj
                    nc.tensor.matmul(bc[st:st + 32, :],
                                     lhsT=negs_s[st + 32:st + 33, 0:32],
                                     rhs=rc_s[st + 32:st + 33, :],
                                     start=True, stop=True,
                                     tile_position=(st + 32, st))
                avs = scratch.tile([P, 512], f32, tag="avs", name="avs")
                nrm = scratch.tile([P, 512], bf16, tag="nrm", name="nrm")
                for jj in range(2):
                    st = 64 * jj
                    nc.scalar.activation(out=avs[st:st + 32, :],
                                         in_=av[st:st + 32, :], func=AF.Copy)
                    nc.vector.tensor_tensor(out=nrm[st:st + 32, :],
                                            in0=avs[st:st + 32, :],
                                            in1=bc[st:st + 32, :],
                                            op=ALU.mult)
                    nc.gpsimd.tensor_tensor(out=ot_s[st:st + 32, pr, qs_],
                                            in0=nrm[st:st + 32, :],
                                            in1=qt_s[st:st + 32, pr, qs_],
                                            op=ALU.add)
            return emit

        def ffn1_chunk(hf):
            cs = slice(hf * 512, hf * 512 + 512)
            for mt in range(DFF // P):
                ms = slice(mt * P, mt * P + P)
                ps = proj_pool.tile([P, 512], f32, tag="proj", name="ps")
                for o in range(4):
                    nc.tensor.matmul(ps[:], lhsT=w1_s[:, o, ms],
                                     rhs=oln_s[:, o, cs],
                                     start=(o == 0), stop=(o == 3))
                nc.scalar.activation(out=h_s[:, mt, cs], in_=ps[:],
                                     func=AF.Gelu, bias=b1_s[:, mt:mt + 1])

        def ffn2_chunk(hf):
            cs = slice(hf * 512, hf * 512 + 512)
            for mt in range(4):
                ms = slice(mt * P, mt * P + P)
                ps = proj_pool.tile([P, 512], f32, tag="proj", name="ps")
                for o in range(4):
                    nc.tensor.matmul(ps[:], lhsT=w2_s[:, o, ms],
                                     rhs=h_s[:, o, cs],
                                     start=(o == 0), stop=False)
                nc.tensor.matmul(ps[:], lhsT=w2_s[0:1, 4, ms],
                                 rhs=ones_s[0:1, 0:512], start=False, stop=True)
                nc.vector.tensor_tensor(out=outt_s[:, mt, cs], in0=ps[:],
                                        in1=ot_s[:, mt, cs], op=ALU.add)
            for h in range(H):
                nc.sync.dma_start(
                    out_d[32 * h:32 * h + 32, cs],
                    outt_s[64 * (h % 2):64 * (h % 2) + 32, h // 2, cs])

        def ln1_square_sums(hf):
            nc.scalar.activation(out=h_s[:, :, hf * 512:hf * 512 + 512],
                                 in_=ot_s[:, :, hf * 512:hf * 512 + 512],
                                 func=AF.Square)
            ln_sums(ot_s, h_s, 4, hf)

        pending = None       # previous iteration's epilogue
        deferred = []        # chunked LN1/FFN stages

        for idx, (qt, pr) in enumerate(
                [(q, p) for q in range(NQT) for p in range(4)]):
            qs_ = slice(qt * 512, qt * 512 + 512)
            av = av_pool.tile([P, 512], f32, tag="av", name="av")
            for kt in range(NKT):
                ks_ = slice(kt * P, kt * P + P)
                sp = scores_pool.tile([P, 1024], f32, tag="scores", name="sp")
                for jj in range(2):
                    h = 2 * pr + jj
                    b, l = 32 * (h % 4), h // 4
                    nc.tensor.matmul(
                        sp[:, jj * 512:jj * 512 + 512],
                        lhsT=kt8[b:b + 16, l, :, ks_],
                        rhs=qt8[b:b + 16, l, :, qs_],
                        start=True, stop=True, perf_mode=DR,
                        tile_position=(b, 0))
                if kt % 2 == 0:
                    ab = apool.tile([P, 1024], bf16, tag="a", name="a")
                    nc.scalar.activation(out=ab[:], in_=sp[:], func=AF.Exp,
                                         scale=SCALE)
                    a = ab[:]
                else:
                    ai = apool.tile([P, 1024], i16, tag="a", name="a")
                    nc.vector.tensor_scalar(
                        out=ai[:], in0=sp[:], scalar1=SCH_A, scalar2=SCH_B,
                        op0=ALU.mult, op1=ALU.add)
                    a = ai[:].bitcast(bf16)
                for jj in range(2):
                    h = 2 * pr + jj
                    st = 64 * jj
                    nc.tensor.matmul(
                        av[st:st + 33, :],
                        lhsT=v_s[:, kt, 33 * h:33 * h + 33],
                        rhs=a[:, jj * 512:jj * 512 + 512],
                        start=(kt == 0), stop=(kt == NKT - 1),
                        tile_position=(0, st),
                        skip_group_check=True)
                if kt == 3 and pending is not None:
                    pending()
                    pending = None
                elif kt in (8, 12) and deferred:
                    deferred.pop(0)()
            pending = make_epilogue(pr, qt, av)
            if idx == 3:
                deferred.extend([
                    lambda: ln1_square_sums(0),
                    lambda: ln_finish(ot_s, oln_s, 4, 0),
                    lambda: ffn1_chunk(0),
                    lambda: ffn2_chunk(0),
                ])
        pending()
        ln1_square_sums(1)
        ln_finish(ot_s, oln_s, 4, 1)
        ffn1_chunk(1)
        ffn2_chunk(1)

    nc.compile()
    return nc


def get_nc():
    if "nc" not in _NC_CACHE:
        _NC_CACHE["nc"] = _build_nc()
    return _NC_CACHE["nc"]


def _host_prep(inputs):
    import ml_dtypes

    bf = ml_dtypes.bfloat16
    f = lambda k: np.asarray(inputs[k], np.float32)
    x, y = f("x"), f("y")
    Wq, bq, Wk, bk, Wv, bv = f("Wq"), f("bq"), f("Wk"), f("bk"), f("Wv"), f("bv")
    W1, b1, W2, b2 = f("W1"), f("b1"), f("W2"), f("b2")
    ln0_g, ln0_b, ln1_g, ln1_b = f("ln0_g"), f("ln0_b"), f("ln1_g"), f("ln1_b")
    # fold LN affines into the following linears; fold bv into bq (sum(A)=1)
    Wq_eff = Wq * ln0_g[None, :]
    bq_eff = bq + Wq @ ln0_b + bv
    W1_eff = W1 * ln1_g[None, :]
    b1_eff = b1 + W1 @ ln1_b

    # permutation: original feature d=32h+i -> slot(h,i) in the 512 space
    slots = np.zeros(D, np.int64)
    for h in range(H):
        for i in range(DH):
            slots[DH * h + i] = _slot(h, i)

    wq_h = np.zeros((D, DSLOT), np.float32)
    wq_h[:, slots] = Wq_eff.T            # [din, dout-slot]
    bq_h = np.zeros(DSLOT, np.float32)
    bq_h[slots] = bq_eff
    wk_h = np.zeros((D, DSLOT), np.float32)
    wk_h[:, slots] = Wk.T
    bk_h = np.zeros(DSLOT, np.float32)
    bk_h[slots] = bk
    wv_h = np.zeros((D, H * 33), np.float32)
    for h in range(H):
        wv_h[:, 33 * h:33 * h + 32] = Wv.T[:, DH * h:DH * h + DH]
    w1_h = np.zeros((DSLOT, DFF), np.float32)
    w1_h[slots, :] = W1_eff.T            # [din-slot, dff]
    w2_h = np.zeros((DFF + 1, DSLOT), np.float32)
    w2_h[0:DFF, slots] = W2.T
    w2_h[DFF, slots] = b2

    wq_h = wq_h.astype(bf)
    wk_h = wk_h.astype(bf)
    wv_h = wv_h.astype(bf)
    w1_h = w1_h.astype(bf)
    w2_h = w2_h.astype(bf)

    in_maps = []
    for core in range(8):
        b, half = core // 2, core % 2
        in_maps.append({
            "xt": np.ascontiguousarray(
                x[b, half * NTOK:(half + 1) * NTOK, :].T).astype(bf),
            "yt": np.ascontiguousarray(y[b].T).astype(bf),
            "wq": wq_h, "bq": bq_h, "wk": wk_h, "bk": bk_h, "wv": wv_h,
            "w1": w1_h, "b1": np.ascontiguousarray(b1_eff), "w2": w2_h,
        })
    return in_maps


def kernel_with_results(inputs, **run_kwargs):
    from concourse.bass_utils import run_bass_kernel_spmd
    nc = get_nc()
    in_maps = _host_prep(inputs)
    res = run_bass_kernel_spmd(nc, in_maps, core_ids=list(range(8)), **run_kwargs)
    out = np.empty((B, N, D), np.float32)
    for core in range(8):
        b, half = core // 2, core % 2
        out[b, half * NTOK:(half + 1) * NTOK, :] = res.results[core]["out_t"].T
    return out, res


def kernel(**inputs):
    out, _ = kernel_with_results(inputs)
    return out


# revision 32
# speedup vs baseline: 1.4718x; 1.4718x over previous
"""Trainium2 Bass kernel for a multi-head self-attention block.

Reference computation (B=4, N=2048, D=256, H=8, dh=32, DFF=512):
    x_ln = LN0(x); Q = x_ln@Wq.T+bq; K = y@Wk.T+bk; V = y@Wv.T+bv
    per head: A = softmax(Qh Kh^T / 16); O = concat_h(Qh + A Vh)
    out = O + (gelu(LN1(O)@W1.T+b1) @ W2.T + b2)

Sharding: 8 cores = 4 batches x 2 halves of the query sequence. Each core
gets its x half-shard and the full y for its batch; no collectives.

Layout: feature-on-partition ("transposed") everywhere. The 256 feature
dims of Q/O are spread over a 512-slot space [128 partitions, 4 ktiles]:
head h lives at partition strip 64*(h%2)..+32, ktile o=h//2. LN folds,
head permutation, and the V-bias fold (bv moves into bq since sum(A)=1)
are host-side weight prep. No max-subtraction in softmax (|s/16|<~1.5).

The attention bmm-softmax-bmm runs in fp8e4m3 with DoubleRow matmuls
(2 contraction rows/cycle): scores contract dh=32 as [16 partitions x 2
subtiles] per head (Q/K relayed out to 16-partition strips by on-chip
DMA), and AV contracts 256 keys (2 key tiles) per matmul with V stored
as [128 tokens, ktpair, 2, head*32]. Softmax denominators come from a
dedicated DoubleRow matmul with a [1|0...] fp8 stationary, landing on
PSUM rows 32/96 of the AV accumulator. Projections/FFN stay bf16; PSUM
accumulation is fp32 throughout.

The exp is split act/DVE by key-tile parity: even tiles get the exact
Act-engine Exp (fp8 out), odd tiles a Schraudolph exp on the DVE (one
tensor_scalar building fp8e4m3 bit patterns in int8; its +-4% equi-
ripple error is common-mode across the softmax and mostly cancels).
Softmax and LN reciprocals run on GPSIMD as a Mitchell bit-trick seed +
one Newton step (standard convert/mult ops; GPSIMD cannot touch PSUM,
so Act first copies the denominator row to SBUF). The seed produces
-1/d; the broadcast matmul's stationary is -1 to flip it back.

Engines have in-order queues, so issue order is the main scheduling
tool: K/V projections are issued inside the LN0 scalar chain, each
attention iteration's epilogue is deferred into the next iteration's
key loop, and LN1/FFN/output-DMA are chunked per 512 tokens and
interleaved into the tail attention iterations.
"""

import contextlib

import numpy as np

B, N, D = 4, 2048, 256
H, DH, DFF = 8, 32, 512
P = 128
NTOK = N // 2            # query tokens per core
NQT = NTOK // 512        # q tiles of 512
NKT = N // P             # key tiles of 128
SCALE = 1.0 / 16.0
EPS = 1e-5
DSLOT = 512              # padded feature-slot space for Q/O

# Schraudolph exp constants for bf16 bit patterns in int16:
#   bits = round(s * SCALE*128*log2(e) + (127*128 - c8))
LOG2E = 1.4426950408889634
SCH_A = SCALE * 128.0 * LOG2E
SCH_B = 127.0 * 128.0 - 366392.5 / 65536.0
# Mitchell reciprocal seed: bits(y0) = C2 - bits(d); one Newton step
# (u-2)*y0 then gives -1/d to ~0.3%.
RCP_C2 = 2129834424.0

_NC_CACHE = {}


def _slot(h, i):
    return (h // 2) * P + 64 * (h % 2) + i


def _build_nc():
    import concourse.mybir as mybir
    import concourse.tile as tile
    from concourse import bacc

    f32 = mybir.dt.float32
    bf16 = mybir.dt.bfloat16
    fp8 = mybir.dt.float8e4
    i16 = mybir.dt.int16
    i32 = mybir.dt.int32
    AF = mybir.ActivationFunctionType
    ALU = mybir.AluOpType
    DR = mybir.MatmulPerfMode.DoubleRow

    nc = bacc.Bacc("TRN2", target_bir_lowering=False, debug=False)

    xt_d = nc.dram_tensor("xt", [D, NTOK], bf16, kind="ExternalInput")
    yt_d = nc.dram_tensor("yt", [D, N], bf16, kind="ExternalInput")
    wq_d = nc.dram_tensor("wq", [D, DSLOT], bf16, kind="ExternalInput")
    bq_d = nc.dram_tensor("bq", [DSLOT], f32, kind="ExternalInput")
    wk_d = nc.dram_tensor("wk", [D, DSLOT], bf16, kind="ExternalInput")
    bk_d = nc.dram_tensor("bk", [DSLOT], f32, kind="ExternalInput")
    wv_d = nc.dram_tensor("wv", [D, H * 33], bf16, kind="ExternalInput")
    w1_d = nc.dram_tensor("w1", [DSLOT, DFF], bf16, kind="ExternalInput")
    b1_d = nc.dram_tensor("b1", [DFF], f32, kind="ExternalInput")
    w2_d = nc.dram_tensor("w2", [DFF + 1, DSLOT], bf16, kind="ExternalInput")
    out_d = nc.dram_tensor("out_t", [D, NTOK], f32, kind="ExternalOutput")

    with tile.TileContext(nc) as tc, contextlib.ExitStack() as ctx:
        const = ctx.enter_context(tc.tile_pool(name="const", bufs=1))
        big = ctx.enter_context(tc.tile_pool(name="big", bufs=1))
        scratch = ctx.enter_context(tc.tile_pool(name="scratch", bufs=1))
        apool = ctx.enter_context(tc.tile_pool(name="apool", bufs=4))
        # PSUM: scores 2x[128,1024]=4 banks, av 2, bc 1, proj 1.
        scores_pool = ctx.enter_context(
            tc.tile_pool(name="scoresp", bufs=2, space="PSUM"))
        av_pool = ctx.enter_context(tc.tile_pool(name="avp", bufs=2, space="PSUM"))
        bc_pool = ctx.enter_context(tc.tile_pool(name="bcp", bufs=1, space="PSUM"))
        proj_pool = ctx.enter_context(tc.tile_pool(name="projp", bufs=1, space="PSUM"))

        # ---- constants / inputs (K-proj inputs first) ---------------------
        yt_s = big.tile([P, 2, N], bf16)
        nc.sync.dma_start(yt_s[:], yt_d.rearrange("(o p) t -> p o t", p=P))
        wk_s = const.tile([P, 2, DSLOT], bf16)
        nc.sync.dma_start(wk_s[:], wk_d.rearrange("(o p) m -> p o m", p=P))
        xt_s = big.tile([P, 2, NTOK], bf16)
        nc.sync.dma_start(xt_s[:], xt_d.rearrange("(o p) t -> p o t", p=P))
        wv_s = const.tile([P, 2, H * 33], bf16)
        nc.sync.dma_start(wv_s[:], wv_d.rearrange("(o p) m -> p o m", p=P))
        wq_s = const.tile([P, 2, DSLOT], bf16)
        nc.sync.dma_start(wq_s[:], wq_d.rearrange("(o p) m -> p o m", p=P))
        w1_s = const.tile([P, 4, DFF], bf16)
        nc.sync.dma_start(w1_s[:], w1_d.rearrange("(o p) m -> p o m", p=P))
        w2_s = const.tile([P, 5, DSLOT], bf16)
        nc.sync.dma_start(w2_s[:, 0:4, :],
                          w2_d[0:DFF, :].rearrange("(o p) m -> p o m", p=P))
        nc.sync.dma_start(w2_s[0:1, 4, :], w2_d[DFF:, :])
        bq_s = const.tile([P, 4], f32)
        nc.sync.dma_start(bq_s[:], bq_d.rearrange("(m p) -> p m", p=P))
        bk_s = const.tile([P, 4], f32)
        nc.sync.dma_start(bk_s[:], bk_d.rearrange("(m p) -> p m", p=P))
        b1_s = const.tile([P, 4], f32)
        nc.sync.dma_start(b1_s[:], b1_d.rearrange("(m p) -> p m", p=P))

        ones_s = const.tile([P, 512], bf16)
        nc.vector.memset(ones_s[:], 1.0)
        negs_s = const.tile([P, 512], bf16)
        nc.vector.memset(negs_s[:], -1.0)
        eps_s = const.tile([1, 1], f32)
        nc.vector.memset(eps_s[:], EPS)


        # ---- shared LN scratch --------------------------------------------
        mean = scratch.tile([1, NTOK], f32, tag="mean")
        mean_b = scratch.tile([1, NTOK], bf16, tag="mean_b")
        rstd_b = scratch.tile([1, NTOK], bf16, tag="rstd_b")
        lt = scratch.tile([1, NTOK], f32, tag="lntmp")
        m2 = scratch.tile([1, NTOK], f32, tag="m2")
        mb_sb = scratch.tile([P, NTOK], bf16, tag="mb_sb")
        rb_sb = scratch.tile([P, NTOK], bf16, tag="rb_sb")
        rsf = scratch.tile([1, NTOK], f32, tag="rsf")

        def ln_sums(src, sq, no, hf):
            """Square already computed into sq; accumulate chunk sums into one
            proj_pool tile (sx at row 0, sq at row 32) -> mean / E[x^2]."""
            cs = slice(hf * 512, hf * 512 + 512)
            ps = proj_pool.tile([P, 512], f32, tag="proj", name="lnsum")
            for o in range(no):
                nc.tensor.matmul(ps[0:1, :], lhsT=ones_s[:, 0:1],
                                 rhs=src[:, o, cs],
                                 start=(o == 0), stop=(o == no - 1),
                                 tile_position=(0, 0), skip_group_check=True)
                nc.tensor.matmul(ps[32:33, :], lhsT=ones_s[:, 0:1],
                                 rhs=sq[:, o, cs],
                                 start=(o == 0), stop=(o == no - 1),
                                 tile_position=(0, 32), skip_group_check=True)
            nc.vector.tensor_scalar_mul(mean[0:1, cs], ps[0:1, :], 1.0 / D)
            nc.vector.tensor_scalar_mul(lt[0:1, cs], ps[32:33, :], 1.0 / D)

        def ln_finish(src, dst, no, hf):
            """rstd for chunk hf, broadcast, normalize src->dst (GPSIMD)."""
            cs = slice(hf * 512, hf * 512 + 512)
            nc.vector.tensor_tensor(out=m2[0:1, cs], in0=mean[0:1, cs],
                                    in1=mean[0:1, cs], op=ALU.mult)
            nc.vector.tensor_tensor(out=lt[0:1, cs], in0=lt[0:1, cs],
                                    in1=m2[0:1, cs], op=ALU.subtract)
            nc.scalar.activation(out=lt[0:1, cs], in_=lt[0:1, cs], func=AF.Sqrt,
                                 bias=eps_s[:])
            nc.vector.reciprocal(out=rsf[0:1, cs], in_=lt[0:1, cs])
            nc.vector.tensor_copy(out=rstd_b[0:1, cs], in_=rsf[0:1, cs])
            nc.vector.tensor_copy(out=mean_b[0:1, cs], in_=mean[0:1, cs])
            br = scores_pool.tile([P, 1024], f32, tag="scores", name="br")
            nc.tensor.matmul(br[:, 0:512], lhsT=ones_s[0:1, 0:P],
                             rhs=mean_b[0:1, cs], start=True, stop=True)
            nc.tensor.matmul(br[:, 512:1024], lhsT=ones_s[0:1, 0:P],
                             rhs=rstd_b[0:1, cs], start=True, stop=True)
            nc.scalar.activation(out=mb_sb[:, cs], in_=br[:, 0:512], func=AF.Copy)
            nc.scalar.activation(out=rb_sb[:, cs], in_=br[:, 512:1024],
                                 func=AF.Copy)
            for o in range(no):
                nc.gpsimd.tensor_tensor(out=dst[:, o, cs], in0=src[:, o, cs],
                                        in1=mb_sb[:, cs], op=ALU.subtract)
                nc.gpsimd.tensor_tensor(out=dst[:, o, cs], in0=dst[:, o, cs],
                                        in1=rb_sb[:, cs], op=ALU.mult)

        # ---- phase A: LN0 (sums), K/V proj, LN0 finish, Q proj -------------
        xln_s = big.tile([P, 2, NTOK], bf16)
        oln_s = big.tile([P, 4, NTOK], bf16)
        sq0 = oln_s[:, 0:2, :]                 # borrow as Square scratch
        nc.scalar.activation(out=sq0[:], in_=xt_s[:], func=AF.Square)
        for hf in range(NQT):
            ln_sums(xt_s, sq0, 2, hf)

        # K proj straight to fp8 (scores are the only K consumer).
        kt8_pre = big.tile([P, 4, N], fp8)
        for mt in range(4):
            for nt in range(N // 512):
                ns_ = slice(nt * 512, nt * 512 + 512)
                ps = proj_pool.tile([P, 512], f32, tag="proj", name="ps")
                for o in range(2):
                    nc.tensor.matmul(ps[:], lhsT=wk_s[:, o, mt * P:mt * P + P],
                                     rhs=yt_s[:, o, ns_],
                                     start=(o == 0), stop=(o == 1))
                if nt % 2 == 0:
                    nc.scalar.activation(out=kt8_pre[:, mt, ns_], in_=ps[:],
                                         func=AF.Identity,
                                         bias=bk_s[:, mt:mt + 1])
                else:
                    nc.vector.tensor_scalar_add(kt8_pre[:, mt, ns_], ps[:],
                                                bk_s[:, mt:mt + 1])
        # relayout to 16-partition dh-subtile strips for DoubleRow scores
        kt8 = big.tile([P, 2, 2, N], fp8)
        for h in range(H):
            b, l = 32 * (h % 4), h // 4
            for s in range(2):
                nc.sync.dma_start(
                    kt8[b:b + 16, l, s, :],
                    kt8_pre[64 * (h % 2) + 16 * s:64 * (h % 2) + 16 * s + 16,
                            h // 2, :])
        # V in natural [token, dout] layout, 33-wide head blocks ([Vh | ones])
        v_s = big.tile([P, NKT, H * 33], bf16)
        for tt in range(NKT):
            ts_ = slice(tt * P, tt * P + P)
            ps = proj_pool.tile([P, 512], f32, tag="proj", name="ps")[:, 0:H * 33]
            for o in range(2):
                nc.tensor.matmul(ps[:], lhsT=yt_s[:, o, ts_],
                                 rhs=wv_s[:, o, :], start=(o == 0), stop=(o == 1))
            if tt % 2 == 0:
                nc.scalar.activation(out=v_s[:, tt, :], in_=ps[:], func=AF.Copy)
            else:
                nc.vector.tensor_copy(out=v_s[:, tt, :], in_=ps[:])
        for h in range(H):
            nc.gpsimd.memset(v_s[:, :, 33 * h + 32], 1.0)

        for hf in range(NQT):
            ln_finish(xt_s, xln_s, 2, hf)

        qt_s = big.tile([P, 4, NTOK], bf16)
        for mt in range(4):
            for nt in range(NQT):
                ns_ = slice(nt * 512, nt * 512 + 512)
                ps = proj_pool.tile([P, 512], f32, tag="proj", name="ps")
                for o in range(2):
                    nc.tensor.matmul(ps[:], lhsT=wq_s[:, o, mt * P:mt * P + P],
                                     rhs=xln_s[:, o, ns_],
                                     start=(o == 0), stop=(o == 1))
                nc.scalar.activation(out=qt_s[:, mt, ns_], in_=ps[:],
                                     func=AF.Identity, bias=bq_s[:, mt:mt + 1])
        qt8_pre = big.tile([P, 4, NTOK], fp8)
        nc.gpsimd.tensor_copy(out=qt8_pre[:], in_=qt_s[:])
        qt8 = big.tile([P, 2, 2, NTOK], fp8)
        for h in range(H):
            b, l = 32 * (h % 4), h // 4
            for s in range(2):
                nc.sync.dma_start(
                    qt8[b:b + 16, l, s, :],
                    qt8_pre[64 * (h % 2) + 16 * s:64 * (h % 2) + 16 * s + 16,
                            h // 2, :])

        # ---- phase B (attention) with phase C (LN1+FFN) interleaved --------
        ot_s = big.tile([P, 4, NTOK], bf16)
        nc.gpsimd.memset(ot_s[32:64, :, :], 0.0)
        nc.gpsimd.memset(ot_s[96:128, :, :], 0.0)
        h_s = big.tile([P, 4, NTOK], bf16)
        outt_s = big.tile([P, 4, NTOK], f32)
        rcf_s = scratch.tile([P, 512], f32, tag="rcf")
        rc_s = scratch.tile([P, 512], bf16, tag="rc")

        def make_epilogue(pr, qt, av):
            qs_ = slice(qt * 512, qt * 512 + 512)

            def emit():
                for jj in range(2):
                    st = 64 * jj
                    nc.vector.reciprocal(out=rcf_s[st + 32:st + 33, :],
                                         in_=av[st + 32:st + 33, :])
                    nc.vector.tensor_copy(out=rc_s[st + 32:st + 33, :],
                                          in_=rcf_s[st + 32:st + 33, :])
                bc = bc_pool.tile([P, 512], f32, tag="bc", name="bc")
                for jj in range(2):
                    st = 64 * jj
                    nc.tensor.matmul(bc[st:st + 32, :],
                                     lhsT=ones_s[st + 32:st + 33, 0:32],
                                     rhs=rc_s[st + 32:st + 33, :],
                                     start=True, stop=True,
                                     tile_position=(st + 32, st))
                avs = scratch.tile([P, 512], f32, tag="avs", name="avs")
                nrm = scratch.tile([P, 512], bf16, tag="nrm", name="nrm")
                for jj in range(2):
                    st = 64 * jj
                    nc.scalar.activation(out=avs[st:st + 32, :],
                                         in_=av[st:st + 32, :], func=AF.Copy)
                    nc.vector.tensor_tensor(out=nrm[st:st + 32, :],
                                            in0=avs[st:st + 32, :],
                                            in1=bc[st:st + 32, :],
                                            op=ALU.mult)
                    nc.gpsimd.tensor_tensor(out=ot_s[st:st + 32, pr, qs_],
                                            in0=nrm[st:st + 32, :],
                                            in1=qt_s[st:st + 32, pr, qs_],
                                            op=ALU.add)
            return emit

        def ffn1_chunk(hf):
            cs = slice(hf * 512, hf * 512 + 512)
            for mt in range(DFF // P):
                ms = slice(mt * P, mt * P + P)
                ps = proj_pool.tile([P, 512], f32, tag="proj", name="ps")
                for o in range(4):
                    nc.tensor.matmul(ps[:], lhsT=w1_s[:, o, ms],
                                     rhs=oln_s[:, o, cs],
                                     start=(o == 0), stop=(o == 3))
                nc.scalar.activation(out=h_s[:, mt, cs], in_=ps[:],
                                     func=AF.Gelu, bias=b1_s[:, mt:mt + 1])

        def ffn2_chunk(hf):
            cs = slice(hf * 512, hf * 512 + 512)
            for mt in range(4):
                ms = slice(mt * P, mt * P + P)
                ps = proj_pool.tile([P, 512], f32, tag="proj", name="ps")
                for o in range(4):
                    nc.tensor.matmul(ps[:], lhsT=w2_s[:, o, ms],
                                     rhs=h_s[:, o, cs],
                                     start=(o == 0), stop=False)
                nc.tensor.matmul(ps[:], lhsT=w2_s[0:1, 4, ms],
                                 rhs=ones_s[0:1, 0:512], start=False, stop=True)
                nc.vector.tensor_tensor(out=outt_s[:, mt, cs], in0=ps[:],
                                        in1=ot_s[:, mt, cs], op=ALU.add)
            for h in range(H):
                nc.sync.dma_start(
                    out_d[32 * h:32 * h + 32, cs],
                    outt_s[64 * (h % 2):64 * (h % 2) + 32, h // 2, cs])

        def ln1_square_sums(hf):
            nc.scalar.activation(out=h_s[:, :, hf * 512:hf * 512 + 512],
                                 in_=ot_s[:, :, hf * 512:hf * 512 + 512],
                                 func=AF.Square)
            ln_sums(ot_s, h_s, 4, hf)

        pending = None       # previous iteration's epilogue
        deferred = []        # chunked LN1/FFN stages

        for idx, (qt, pr) in enumerate(
                [(q, p) for q in range(NQT) for p in range(4)]):
            qs_ = slice(qt * 512, qt * 512 + 512)
            av = av_pool.tile([P, 512], f32, tag="av", name="av")

            def av_mm(kt, a):
                # AV for key tile kt, issued one kt late so it never blocks
                # the next score matmuls at the head of the in-order PE queue
                for jj in range(2):
                    h = 2 * pr + jj
                    st = 64 * jj
                    nc.tensor.matmul(
                        av[st:st + 33, :],
                        lhsT=v_s[:, kt, 33 * h:33 * h + 33],
                        rhs=a[:, jj * 512:jj * 512 + 512],
                        start=(kt == 0), stop=(kt == NKT - 1),
                        tile_position=(0, st),
                        skip_group_check=True)

            prev_av = None
            for kt in range(NKT):
                ks_ = slice(kt * P, kt * P + P)
                sp = scores_pool.tile([P, 1024], f32, tag="scores", name="sp")
                for jj in range(2):
                    h = 2 * pr + jj
                    b, l = 32 * (h % 4), h // 4
                    nc.tensor.matmul(
                        sp[:, jj * 512:jj * 512 + 512],
                        lhsT=kt8[b:b + 16, l, :, ks_],
                        rhs=qt8[b:b + 16, l, :, qs_],
                        start=True, stop=True, perf_mode=DR,
                        tile_position=(b, 0))
                if kt % 4 != 1:
                    ab = apool.tile([P, 1024], bf16, tag="a", name="a")
                    nc.scalar.activation(out=ab[:], in_=sp[:], func=AF.Exp,
                                         scale=SCALE)
                    a = ab[:]
                else:
                    ai = apool.tile([P, 1024], i16, tag="a", name="a")
                    nc.vector.tensor_scalar(
                        out=ai[:], in0=sp[:], scalar1=SCH_A, scalar2=SCH_B,
                        op0=ALU.mult, op1=ALU.add)
                    a = ai[:].bitcast(bf16)
                if prev_av is not None:
                    av_mm(kt - 1, prev_av)
                prev_av = a
                if kt == 3 and pending is not None:
                    pending()
                    pending = None
                elif kt in (8, 12) and deferred:
                    deferred.pop(0)()
            av_mm(NKT - 1, prev_av)
            pending = make_epilogue(pr, qt, av)
            if idx == 3:
                deferred.extend([
                    lambda: ln1_square_sums(0),
                    lambda: ln_finish(ot_s, oln_s, 4, 0),
                    lambda: ffn1_chunk(0),
                    lambda: ffn2_chunk(0),
                ])
        pending()
        ln1_square_sums(1)
        ln_finish(ot_s, oln_s, 4, 1)
        ffn1_chunk(1)
        ffn2_chunk(1)

    nc.compile()
    return nc


def get_nc():
    if "nc" not in _NC_CACHE:
        _NC_CACHE["nc"] = _build_nc()
    return _NC_CACHE["nc"]


def _host_prep(inputs):
    import ml_dtypes

    bf = ml_dtypes.bfloat16
    f = lambda k: np.asarray(inputs[k], np.float32)
    x, y = f("x"), f("y")
    Wq, bq, Wk, bk, Wv, bv = f("Wq"), f("bq"), f("Wk"), f("bk"), f("Wv"), f("bv")
    W1, b1, W2, b2 = f("W1"), f("b1"), f("W2"), f("b2")
    ln0_g, ln0_b, ln1_g, ln1_b = f("ln0_g"), f("ln0_b"), f("ln1_g"), f("ln1_b")
    # fold LN affines into the following linears; fold bv into bq (sum(A)=1)
    Wq_eff = Wq * ln0_g[None, :]
    bq_eff = bq + Wq @ ln0_b + bv
    W1_eff = W1 * ln1_g[None, :]
    b1_eff = b1 + W1 @ ln1_b

    # permutation: original feature d=32h+i -> slot(h,i) in the 512 space
    slots = np.zeros(D, np.int64)
    for h in range(H):
        for i in range(DH):
            slots[DH * h + i] = _slot(h, i)

    wq_h = np.zeros((D, DSLOT), np.float32)
    wq_h[:, slots] = Wq_eff.T            # [din, dout-slot]
    bq_h = np.zeros(DSLOT, np.float32)
    bq_h[slots] = bq_eff
    wk_h = np.zeros((D, DSLOT), np.float32)
    wk_h[:, slots] = Wk.T
    bk_h = np.zeros(DSLOT, np.float32)
    bk_h[slots] = bk
    wv_h = np.zeros((D, H * 33), np.float32)
    for h in range(H):
        wv_h[:, 33 * h:33 * h + 32] = Wv.T[:, DH * h:DH * h + DH]
    w1_h = np.zeros((DSLOT, DFF), np.float32)
    w1_h[slots, :] = W1_eff.T            # [din-slot, dff]
    w2_h = np.zeros((DFF + 1, DSLOT), np.float32)
    w2_h[0:DFF, slots] = W2.T
    w2_h[DFF, slots] = b2

    wq_h = wq_h.astype(bf)
    wk_h = wk_h.astype(bf)
    wv_h = wv_h.astype(bf)
    w1_h = w1_h.astype(bf)
    w2_h = w2_h.astype(bf)

    in_maps = []
    for core in range(8):
        b, half = core // 2, core % 2
        in_maps.append({
            "xt": np.ascontiguousarray(
                x[b, half * NTOK:(half + 1) * NTOK, :].T).astype(bf),
            "yt": np.ascontiguousarray(y[b].T).astype(bf),
            "wq": wq_h, "bq": bq_h, "wk": wk_h, "bk": bk_h, "wv": wv_h,
            "w1": w1_h, "b1": np.ascontiguousarray(b1_eff), "w2": w2_h,
        })
    return in_maps


def kernel_with_results(inputs, **run_kwargs):
    from concourse.bass_utils import run_bass_kernel_spmd
    nc = get_nc()
    in_maps = _host_prep(inputs)
    res = run_bass_kernel_spmd(nc, in_maps, core_ids=list(range(8)), **run_kwargs)
    out = np.empty((B, N, D), np.float32)
    for core in range(8):
        b, half = core // 2, core % 2
        out[b, half * NTOK:(half + 1) * NTOK, :] = res.results[core]["out_t"].T
    return out, res


def kernel(**inputs):
    out, _ = kernel_with_results(inputs)
    return out


# revision 34
# speedup vs baseline: 1.5498x; 1.0530x over previous
"""Trainium2 Bass kernel for a multi-head self-attention block.

Reference computation (B=4, N=2048, D=256, H=8, dh=32, DFF=512):
    x_ln = LN0(x); Q = x_ln@Wq.T+bq; K = y@Wk.T+bk; V = y@Wv.T+bv
    per head: A = softmax(Qh Kh^T / 16); O = concat_h(Qh + A Vh)
    out = O + (gelu(LN1(O)@W1.T+b1) @ W2.T + b2)

Sharding: 8 cores = 4 batches x 2 halves of the query sequence. Each core
gets its x half-shard and the full y for its batch; no collectives.

Layout: feature-on-partition ("transposed") everywhere. The 256 feature
dims of Q/O are spread over a 512-slot space [128 partitions, 4 ktiles]:
head h lives at partition strip 64*(h%2)..+32, ktile o=h//2. LN folds,
head permutation, and the V-bias fold (bv moves into bq since sum(A)=1)
are host-side weight prep. No max-subtraction in softmax (|s/16|<~1.5).

The attention bmm-softmax-bmm runs in fp8e4m3 with DoubleRow matmuls
(2 contraction rows/cycle): scores contract dh=32 as [16 partitions x 2
subtiles] per head (Q/K relayed out to 16-partition strips by on-chip
DMA), and AV contracts 256 keys (2 key tiles) per matmul with V stored
as [128 tokens, ktpair, 2, head*32]. Softmax denominators come from a
dedicated DoubleRow matmul with a [1|0...] fp8 stationary, landing on
PSUM rows 32/96 of the AV accumulator. Projections/FFN stay bf16; PSUM
accumulation is fp32 throughout.

The exp is split act/DVE by key-tile parity: even tiles get the exact
Act-engine Exp (fp8 out), odd tiles a Schraudolph exp on the DVE (one
tensor_scalar building fp8e4m3 bit patterns in int8; its +-4% equi-
ripple error is common-mode across the softmax and mostly cancels).
Softmax and LN reciprocals run on GPSIMD as a Mitchell bit-trick seed +
one Newton step (standard convert/mult ops; GPSIMD cannot touch PSUM,
so Act first copies the denominator row to SBUF). The seed produces
-1/d; the broadcast matmul's stationary is -1 to flip it back.

Engines have in-order queues, so issue order is the main scheduling
tool: K/V projections are issued inside the LN0 scalar chain, each
attention iteration's epilogue is deferred into the next iteration's
key loop, and LN1/FFN/output-DMA are chunked per 512 tokens and
interleaved into the tail attention iterations.
"""

import contextlib

import numpy as np

B, N, D = 4, 2048, 256
H, DH, DFF = 8, 32, 512
P = 128
NTOK = N // 2            # query tokens per core
NQT = NTOK // 512        # q tiles of 512
NKT = N // P             # key tiles of 128
SCALE = 1.0 / 16.0
EPS = 1e-5
DSLOT = 512              # padded feature-slot space for Q/O

# Schraudolph exp constants for bf16 bit patterns in int16:
#   bits = round(s * SCALE*128*log2(e) + (127*128 - c8))
LOG2E = 1.4426950408889634
SCH_A = SCALE * 128.0 * LOG2E
SCH_B = 127.0 * 128.0 - 366392.5 / 65536.0
# Mitchell reciprocal seed: bits(y0) = C2 - bits(d); one Newton step
# (u-2)*y0 then gives -1/d to ~0.3%.
RCP_C2 = 2129834424.0

_NC_CACHE = {}


def _slot(h, i):
    return (h // 2) * P + 64 * (h % 2) + i


def _build_nc():
    import concourse.mybir as mybir
    import concourse.tile as tile
    from concourse import bacc

    f32 = mybir.dt.float32
    bf16 = mybir.dt.bfloat16
    fp8 = mybir.dt.float8e4
    i16 = mybir.dt.int16
    i32 = mybir.dt.int32
    AF = mybir.ActivationFunctionType
    ALU = mybir.AluOpType
    DR = mybir.MatmulPerfMode.DoubleRow

    nc = bacc.Bacc("TRN2", target_bir_lowering=False, debug=False)

    xt_d = nc.dram_tensor("xt", [D, NTOK], bf16, kind="ExternalInput")
    yt_d = nc.dram_tensor("yt", [D, N], bf16, kind="ExternalInput")
    wq_d = nc.dram_tensor("wq", [D, DSLOT], bf16, kind="ExternalInput")
    bq_d = nc.dram_tensor("bq", [DSLOT], f32, kind="ExternalInput")
    wk_d = nc.dram_tensor("wk", [D, DSLOT], bf16, kind="ExternalInput")
    bk_d = nc.dram_tensor("bk", [DSLOT], f32, kind="ExternalInput")
    wv_d = nc.dram_tensor("wv", [D, H * 33], bf16, kind="ExternalInput")
    w1_d = nc.dram_tensor("w1", [DSLOT, DFF], bf16, kind="ExternalInput")
    b1_d = nc.dram_tensor("b1", [DFF], f32, kind="ExternalInput")
    w2_d = nc.dram_tensor("w2", [DFF + 1, DSLOT], bf16, kind="ExternalInput")
    out_d = nc.dram_tensor("out_t", [D, NTOK], f32, kind="ExternalOutput")

    with tile.TileContext(nc) as tc, contextlib.ExitStack() as ctx:
        const = ctx.enter_context(tc.tile_pool(name="const", bufs=1))
        big = ctx.enter_context(tc.tile_pool(name="big", bufs=1))
        scratch = ctx.enter_context(tc.tile_pool(name="scratch", bufs=1))
        apool = ctx.enter_context(tc.tile_pool(name="apool", bufs=4))
        # PSUM: scores 2x[128,1024]=4 banks, av 2, bc 1, proj 1.
        scores_pool = ctx.enter_context(
            tc.tile_pool(name="scoresp", bufs=2, space="PSUM"))
        av_pool = ctx.enter_context(tc.tile_pool(name="avp", bufs=2, space="PSUM"))
        bc_pool = ctx.enter_context(tc.tile_pool(name="bcp", bufs=1, space="PSUM"))
        proj_pool = ctx.enter_context(tc.tile_pool(name="projp", bufs=1, space="PSUM"))

        # ---- constants / inputs (K-proj inputs first) ---------------------
        yt_s = big.tile([P, 2, N], bf16)
        nc.sync.dma_start(yt_s[:], yt_d.rearrange("(o p) t -> p o t", p=P))
        wk_s = const.tile([P, 2, DSLOT], bf16)
        nc.sync.dma_start(wk_s[:], wk_d.rearrange("(o p) m -> p o m", p=P))
        xt_s = big.tile([P, 2, NTOK], bf16)
        nc.sync.dma_start(xt_s[:], xt_d.rearrange("(o p) t -> p o t", p=P))
        wv_s = const.tile([P, 2, H * 33], bf16)
        nc.sync.dma_start(wv_s[:], wv_d.rearrange("(o p) m -> p o m", p=P))
        wq_s = const.tile([P, 2, DSLOT], bf16)
        nc.sync.dma_start(wq_s[:], wq_d.rearrange("(o p) m -> p o m", p=P))
        w1_s = const.tile([P, 4, DFF], bf16)
        nc.sync.dma_start(w1_s[:], w1_d.rearrange("(o p) m -> p o m", p=P))
        w2_s = const.tile([P, 5, DSLOT], bf16)
        nc.sync.dma_start(w2_s[:, 0:4, :],
                          w2_d[0:DFF, :].rearrange("(o p) m -> p o m", p=P))
        nc.sync.dma_start(w2_s[0:1, 4, :], w2_d[DFF:, :])
        bq_s = const.tile([P, 4], f32)
        nc.sync.dma_start(bq_s[:], bq_d.rearrange("(m p) -> p m", p=P))
        bk_s = const.tile([P, 4], f32)
        nc.sync.dma_start(bk_s[:], bk_d.rearrange("(m p) -> p m", p=P))
        b1_s = const.tile([P, 4], f32)
        nc.sync.dma_start(b1_s[:], b1_d.rearrange("(m p) -> p m", p=P))

        ones_s = const.tile([P, 512], bf16)
        nc.vector.memset(ones_s[:], 1.0)
        negs_s = const.tile([P, 512], bf16)
        nc.vector.memset(negs_s[:], -1.0)
        eps_s = const.tile([1, 1], f32)
        nc.vector.memset(eps_s[:], EPS)


        # ---- shared LN scratch --------------------------------------------
        mean = scratch.tile([1, NTOK], f32, tag="mean")
        mean_b = scratch.tile([1, NTOK], bf16, tag="mean_b")
        rstd_b = scratch.tile([1, NTOK], bf16, tag="rstd_b")
        lt = scratch.tile([1, NTOK], f32, tag="lntmp")
        m2 = scratch.tile([1, NTOK], f32, tag="m2")
        mb_sb = scratch.tile([P, NTOK], bf16, tag="mb_sb")
        rb_sb = scratch.tile([P, NTOK], bf16, tag="rb_sb")
        rsf = scratch.tile([1, NTOK], f32, tag="rsf")

        def ln_sums(src, sq, no, hf):
            """Square already computed into sq; accumulate chunk sums into one
            proj_pool tile (sx at row 0, sq at row 32) -> mean / E[x^2]."""
            cs = slice(hf * 512, hf * 512 + 512)
            ps = proj_pool.tile([P, 512], f32, tag="proj", name="lnsum")
            for o in range(no):
                nc.tensor.matmul(ps[0:1, :], lhsT=ones_s[:, 0:1],
                                 rhs=src[:, o, cs],
                                 start=(o == 0), stop=(o == no - 1),
                                 tile_position=(0, 0), skip_group_check=True)
                nc.tensor.matmul(ps[32:33, :], lhsT=ones_s[:, 0:1],
                                 rhs=sq[:, o, cs],
                                 start=(o == 0), stop=(o == no - 1),
                                 tile_position=(0, 32), skip_group_check=True)
            nc.vector.tensor_scalar_mul(mean[0:1, cs], ps[0:1, :], 1.0 / D)
            nc.vector.tensor_scalar_mul(lt[0:1, cs], ps[32:33, :], 1.0 / D)

        def ln_finish(src, dst, no, hf):
            """rstd for chunk hf, broadcast, normalize src->dst (GPSIMD)."""
            cs = slice(hf * 512, hf * 512 + 512)
            nc.vector.tensor_tensor(out=m2[0:1, cs], in0=mean[0:1, cs],
                                    in1=mean[0:1, cs], op=ALU.mult)
            nc.vector.tensor_tensor(out=lt[0:1, cs], in0=lt[0:1, cs],
                                    in1=m2[0:1, cs], op=ALU.subtract)
            nc.scalar.activation(out=lt[0:1, cs], in_=lt[0:1, cs], func=AF.Sqrt,
                                 bias=eps_s[:])
            nc.vector.reciprocal(out=rsf[0:1, cs], in_=lt[0:1, cs])
            nc.vector.tensor_copy(out=rstd_b[0:1, cs], in_=rsf[0:1, cs])
            nc.vector.tensor_copy(out=mean_b[0:1, cs], in_=mean[0:1, cs])
            br = scores_pool.tile([P, 1024], f32, tag="scores", name="br")
            nc.tensor.matmul(br[:, 0:512], lhsT=ones_s[0:1, 0:P],
                             rhs=mean_b[0:1, cs], start=True, stop=True)
            nc.tensor.matmul(br[:, 512:1024], lhsT=ones_s[0:1, 0:P],
                             rhs=rstd_b[0:1, cs], start=True, stop=True)
            nc.scalar.activation(out=mb_sb[:, cs], in_=br[:, 0:512], func=AF.Copy)
            nc.scalar.activation(out=rb_sb[:, cs], in_=br[:, 512:1024],
                                 func=AF.Copy)
            for o in range(no):
                nc.gpsimd.tensor_tensor(out=dst[:, o, cs], in0=src[:, o, cs],
                                        in1=mb_sb[:, cs], op=ALU.subtract)
                nc.gpsimd.tensor_tensor(out=dst[:, o, cs], in0=dst[:, o, cs],
                                        in1=rb_sb[:, cs], op=ALU.mult)

        # ---- phase A: LN0 (sums), K/V proj, LN0 finish, Q proj -------------
        xln_s = big.tile([P, 2, NTOK], bf16)
        oln_s = big.tile([P, 4, NTOK], bf16)
        sq0 = oln_s[:, 0:2, :]                 # borrow as Square scratch
        nc.scalar.activation(out=sq0[:], in_=xt_s[:], func=AF.Square)
        for hf in range(NQT):
            ln_sums(xt_s, sq0, 2, hf)

        # K proj straight to fp8 (scores are the only K consumer). The LN0
        # finish chain is issued after two K columns: its broadcast matmuls
        # then sit mid-queue on the PE and their DVE/act inputs are ready by
        # the time the PE drains the remaining K/V work ahead of them.
        kt8_pre = big.tile([P, 4, N], fp8)
        for mt in range(4):
            if mt == 2:
                for hf in range(NQT):
                    ln_finish(xt_s, xln_s, 2, hf)
            for nt in range(N // 512):
                ns_ = slice(nt * 512, nt * 512 + 512)
                ps = proj_pool.tile([P, 512], f32, tag="proj", name="ps")
                for o in range(2):
                    nc.tensor.matmul(ps[:], lhsT=wk_s[:, o, mt * P:mt * P + P],
                                     rhs=yt_s[:, o, ns_],
                                     start=(o == 0), stop=(o == 1))
                if nt % 2 == 0:
                    nc.scalar.activation(out=kt8_pre[:, mt, ns_], in_=ps[:],
                                         func=AF.Identity,
                                         bias=bk_s[:, mt:mt + 1])
                else:
                    nc.vector.tensor_scalar_add(kt8_pre[:, mt, ns_], ps[:],
                                                bk_s[:, mt:mt + 1])
        # relayout to 16-partition dh-subtile strips for DoubleRow scores
        kt8 = big.tile([P, 2, 2, N], fp8)
        for h in range(H):
            b, l = 32 * (h % 4), h // 4
            for s in range(2):
                nc.sync.dma_start(
                    kt8[b:b + 16, l, s, :],
                    kt8_pre[64 * (h % 2) + 16 * s:64 * (h % 2) + 16 * s + 16,
                            h // 2, :])
        # V in natural [token, dout] layout, 33-wide head blocks ([Vh | ones])
        v_s = big.tile([P, NKT, H * 33], bf16)
        for tt in range(NKT):
            ts_ = slice(tt * P, tt * P + P)
            ps = proj_pool.tile([P, 512], f32, tag="proj", name="ps")[:, 0:H * 33]
            for o in range(2):
                nc.tensor.matmul(ps[:], lhsT=yt_s[:, o, ts_],
                                 rhs=wv_s[:, o, :], start=(o == 0), stop=(o == 1))
            if tt % 2 == 0:
                nc.scalar.activation(out=v_s[:, tt, :], in_=ps[:], func=AF.Copy)
            else:
                nc.vector.tensor_copy(out=v_s[:, tt, :], in_=ps[:])
        for h in range(H):
            nc.gpsimd.memset(v_s[:, :, 33 * h + 32], 1.0)

        qt_s = big.tile([P, 4, NTOK], bf16)
        qt8_pre = big.tile([P, 4, NTOK], fp8)
        for mt in range(4):
            for nt in range(NQT):
                ns_ = slice(nt * 512, nt * 512 + 512)
                ps = proj_pool.tile([P, 512], f32, tag="proj", name="ps")
                for o in range(2):
                    nc.tensor.matmul(ps[:], lhsT=wq_s[:, o, mt * P:mt * P + P],
                                     rhs=xln_s[:, o, ns_],
                                     start=(o == 0), stop=(o == 1))
                nc.scalar.activation(out=qt_s[:, mt, ns_], in_=ps[:],
                                     func=AF.Identity, bias=bq_s[:, mt:mt + 1])
                nc.vector.tensor_scalar_add(qt8_pre[:, mt, ns_], ps[:],
                                            bq_s[:, mt:mt + 1])
        qt8 = big.tile([P, 2, 2, NTOK], fp8)
        for h in range(H):
            b, l = 32 * (h % 4), h // 4
            for s in range(2):
                nc.sync.dma_start(
                    qt8[b:b + 16, l, s, :],
                    qt8_pre[64 * (h % 2) + 16 * s:64 * (h % 2) + 16 * s + 16,
                            h // 2, :])

        # ---- phase B (attention) with phase C (LN1+FFN) interleaved --------
        ot_s = big.tile([P, 4, NTOK], bf16)
        nc.gpsimd.memset(ot_s[32:64, :, :], 0.0)
        nc.gpsimd.memset(ot_s[96:128, :, :], 0.0)
        h_s = big.tile([P, 4, NTOK], bf16)
        outt_s = big.tile([P, 4, NTOK], f32)
        rcf_s = scratch.tile([P, 512], f32, tag="rcf")
        rc_s = scratch.tile([P, 512], bf16, tag="rc")

        def make_epilogue(pr, qt, av):
            qs_ = slice(qt * 512, qt * 512 + 512)

            def emit():
                for jj in range(2):
                    st = 64 * jj
                    nc.vector.reciprocal(out=rcf_s[st + 32:st + 33, :],
                                         in_=av[st + 32:st + 33, :])
                    nc.vector.tensor_copy(out=rc_s[st + 32:st + 33, :],
                                          in_=rcf_s[st + 32:st + 33, :])
                bc = bc_pool.tile([P, 512], f32, tag="bc", name="bc")
                for jj in range(2):
                    st = 64 * jj
                    nc.tensor.matmul(bc[st:st + 32, :],
                                     lhsT=ones_s[st + 32:st + 33, 0:32],
                                     rhs=rc_s[st + 32:st + 33, :],
                                     start=True, stop=True,
                                     tile_position=(st + 32, st))
                avs = scratch.tile([P, 512], f32, tag="avs", name="avs")
                nrm = scratch.tile([P, 512], bf16, tag="nrm", name="nrm")
                for jj in range(2):
                    st = 64 * jj
                    nc.scalar.activation(out=avs[st:st + 32, :],
                                         in_=av[st:st + 32, :], func=AF.Copy)
                    nc.vector.tensor_tensor(out=nrm[st:st + 32, :],
                                            in0=avs[st:st + 32, :],
                                            in1=bc[st:st + 32, :],
                                            op=ALU.mult)
                    nc.gpsimd.tensor_tensor(out=ot_s[st:st + 32, pr, qs_],
                                            in0=nrm[st:st + 32, :],
                                            in1=qt_s[st:st + 32, pr, qs_],
                                            op=ALU.add)
            return emit

        def ffn1_chunk(hf):
            cs = slice(hf * 512, hf * 512 + 512)
            for mt in range(DFF // P):
                ms = slice(mt * P, mt * P + P)
                ps = proj_pool.tile([P, 512], f32, tag="proj", name="ps")
                for o in range(4):
                    nc.tensor.matmul(ps[:], lhsT=w1_s[:, o, ms],
                                     rhs=oln_s[:, o, cs],
                                     start=(o == 0), stop=(o == 3))
                nc.scalar.activation(out=h_s[:, mt, cs], in_=ps[:],
                                     func=AF.Gelu, bias=b1_s[:, mt:mt + 1])

        def ffn2_chunk(hf):
            cs = slice(hf * 512, hf * 512 + 512)
            for mt in range(4):
                ms = slice(mt * P, mt * P + P)
                ps = proj_pool.tile([P, 512], f32, tag="proj", name="ps")
                for o in range(4):
                    nc.tensor.matmul(ps[:], lhsT=w2_s[:, o, ms],
                                     rhs=h_s[:, o, cs],
                                     start=(o == 0), stop=False)
                nc.tensor.matmul(ps[:], lhsT=w2_s[0:1, 4, ms],
                                 rhs=ones_s[0:1, 0:512], start=False, stop=True)
                nc.vector.tensor_tensor(out=outt_s[:, mt, cs], in0=ps[:],
                                        in1=ot_s[:, mt, cs], op=ALU.add)
            for h in range(H):
                nc.sync.dma_start(
                    out_d[32 * h:32 * h + 32, cs],
                    outt_s[64 * (h % 2):64 * (h % 2) + 32, h // 2, cs])

        def ln1_square_sums(hf):
            nc.scalar.activation(out=h_s[:, :, hf * 512:hf * 512 + 512],
                                 in_=ot_s[:, :, hf * 512:hf * 512 + 512],
                                 func=AF.Square)
            ln_sums(ot_s, h_s, 4, hf)

        pending = None       # previous iteration's epilogue
        deferred = []        # chunked LN1/FFN stages

        for idx, (qt, pr) in enumerate(
                [(q, p) for q in range(NQT) for p in range(4)]):
            qs_ = slice(qt * 512, qt * 512 + 512)
            av = av_pool.tile([P, 512], f32, tag="av", name="av")

            def av_mm(kt, a):
                # AV for key tile kt, issued one kt late so it never blocks
                # the next score matmuls at the head of the in-order PE queue
                for jj in range(2):
                    h = 2 * pr + jj
                    st = 64 * jj
                    nc.tensor.matmul(
                        av[st:st + 33, :],
                        lhsT=v_s[:, kt, 33 * h:33 * h + 33],
                        rhs=a[:, jj * 512:jj * 512 + 512],
                        start=(kt == 0), stop=(kt == NKT - 1),
                        tile_position=(0, st),
                        skip_group_check=True)

            prev_av = None
            for kt in range(NKT):
                ks_ = slice(kt * P, kt * P + P)
                sp = scores_pool.tile([P, 1024], f32, tag="scores", name="sp")
                for jj in range(2):
                    h = 2 * pr + jj
                    b, l = 32 * (h % 4), h // 4
                    nc.tensor.matmul(
                        sp[:, jj * 512:jj * 512 + 512],
                        lhsT=kt8[b:b + 16, l, :, ks_],
                        rhs=qt8[b:b + 16, l, :, qs_],
                        start=True, stop=True, perf_mode=DR,
                        tile_position=(b, 0))
                if kt % 4 != 1:
                    ab = apool.tile([P, 1024], bf16, tag="a", name="a")
                    nc.scalar.activation(out=ab[:], in_=sp[:], func=AF.Exp,
                                         scale=SCALE)
                    a = ab[:]
                else:
                    ai = apool.tile([P, 1024], i16, tag="a", name="a")
                    nc.vector.tensor_scalar(
                        out=ai[:], in0=sp[:], scalar1=SCH_A, scalar2=SCH_B,
                        op0=ALU.mult, op1=ALU.add)
                    a = ai[:].bitcast(bf16)
                if prev_av is not None:
                    av_mm(kt - 1, prev_av)
                prev_av = a
                if kt == 3 and pending is not None:
                    pending()
                    pending = None
                elif kt in (8, 12) and deferred:
                    deferred.pop(0)()
            av_mm(NKT - 1, prev_av)
            pending = make_epilogue(pr, qt, av)
            if idx == 3:
                deferred.extend([
                    lambda: ln1_square_sums(0),
                    lambda: ln_finish(ot_s, oln_s, 4, 0),
                    lambda: ffn1_chunk(0),
                    lambda: ffn2_chunk(0),
                ])
        pending()
        ln1_square_sums(1)
        ln_finish(ot_s, oln_s, 4, 1)
        ffn1_chunk(1)
        ffn2_chunk(1)

    nc.compile()
    return nc


def get_nc():
    if "nc" not in _NC_CACHE:
        _NC_CACHE["nc"] = _build_nc()
    return _NC_CACHE["nc"]


def _host_prep(inputs):
    import ml_dtypes

    bf = ml_dtypes.bfloat16
    f = lambda k: np.asarray(inputs[k], np.float32)
    x, y = f("x"), f("y")
    Wq, bq, Wk, bk, Wv, bv = f("Wq"), f("bq"), f("Wk"), f("bk"), f("Wv"), f("bv")
    W1, b1, W2, b2 = f("W1"), f("b1"), f("W2"), f("b2")
    ln0_g, ln0_b, ln1_g, ln1_b = f("ln0_g"), f("ln0_b"), f("ln1_g"), f("ln1_b")
    # fold LN affines into the following linears; fold bv into bq (sum(A)=1)
    Wq_eff = Wq * ln0_g[None, :]
    bq_eff = bq + Wq @ ln0_b + bv
    W1_eff = W1 * ln1_g[None, :]
    b1_eff = b1 + W1 @ ln1_b

    # permutation: original feature d=32h+i -> slot(h,i) in the 512 space
    slots = np.zeros(D, np.int64)
    for h in range(H):
        for i in range(DH):
            slots[DH * h + i] = _slot(h, i)

    wq_h = np.zeros((D, DSLOT), np.float32)
    wq_h[:, slots] = Wq_eff.T            # [din, dout-slot]
    bq_h = np.zeros(DSLOT, np.float32)
    bq_h[slots] = bq_eff
    wk_h = np.zeros((D, DSLOT), np.float32)
    wk_h[:, slots] = Wk.T
    bk_h = np.zeros(DSLOT, np.float32)
    bk_h[slots] = bk
    wv_h = np.zeros((D, H * 33), np.float32)
    for h in range(H):
        wv_h[:, 33 * h:33 * h + 32] = Wv.T[:, DH * h:DH * h + DH]
    w1_h = np.zeros((DSLOT, DFF), np.float32)
    w1_h[slots, :] = W1_eff.T            # [din-slot, dff]
    w2_h = np.zeros((DFF + 1, DSLOT), np.float32)
    w2_h[0:DFF, slots] = W2.T
    w2_h[DFF, slots] = b2

    wq_h = wq_h.astype(bf)
    wk_h = wk_h.astype(bf)
    wv_h = wv_h.astype(bf)
    w1_h = w1_h.astype(bf)
    w2_h = w2_h.astype(bf)

    in_maps = []
    for core in range(8):
        b, half = core // 2, core % 2
        in_maps.append({
            "xt": np.ascontiguousarray(
                x[b, half * NTOK:(half + 1) * NTOK, :].T).astype(bf),
            "yt": np.ascontiguousarray(y[b].T).astype(bf),
            "wq": wq_h, "bq": bq_h, "wk": wk_h, "bk": bk_h, "wv": wv_h,
            "w1": w1_h, "b1": np.ascontiguousarray(b1_eff), "w2": w2_h,
        })
    return in_maps


def kernel_with_results(inputs, **run_kwargs):
    from concourse.bass_utils import run_bass_kernel_spmd
    nc = get_nc()
    in_maps = _host_prep(inputs)
    res = run_bass_kernel_spmd(nc, in_maps, core_ids=list(range(8)), **run_kwargs)
    out = np.empty((B, N, D), np.float32)
    for core in range(8):
        b, half = core // 2, core % 2
        out[b, half * NTOK:(half + 1) * NTOK, :] = res.results[core]["out_t"].T
    return out, res


def kernel(**inputs):
    out, _ = kernel_with_results(inputs)
    return out


# revision 43
# speedup vs baseline: 1.6142x; 1.0415x over previous
"""Trainium2 Bass kernel for a multi-head self-attention block.

Reference computation (B=4, N=2048, D=256, H=8, dh=32, DFF=512):
    x_ln = LN0(x); Q = x_ln@Wq.T+bq; K = y@Wk.T+bk; V = y@Wv.T+bv
    per head: A = softmax(Qh Kh^T / 16); O = concat_h(Qh + A Vh)
    out = O + (gelu(LN1(O)@W1.T+b1) @ W2.T + b2)

Sharding: 8 cores = 4 batches x 2 halves of the query sequence. Each core
gets its x half-shard and the full y for its batch; no collectives.

Layout: feature-on-partition ("transposed") everywhere. The 256 feature
dims of Q/O are spread over a 512-slot space [128 partitions, 4 ktiles]:
head h lives at partition strip 64*(h%2)..+32, ktile o=h//2. LN folds,
head permutation, and the V-bias fold (bv moves into bq since sum(A)=1)
are host-side weight prep. No max-subtraction in softmax (|s/16|<~1.5).

The attention bmm-softmax-bmm runs in fp8e4m3 with DoubleRow matmuls
(2 contraction rows/cycle): scores contract dh=32 as [16 partitions x 2
subtiles] per head (Q/K relayed out to 16-partition strips by on-chip
DMA), and AV contracts 256 keys (2 key tiles) per matmul with V stored
as [128 tokens, ktpair, 2, head*32]. Softmax denominators come from a
dedicated DoubleRow matmul with a [1|0...] fp8 stationary, landing on
PSUM rows 32/96 of the AV accumulator. Projections/FFN stay bf16; PSUM
accumulation is fp32 throughout.

The exp is split act/DVE by key-tile parity: even tiles get the exact
Act-engine Exp (fp8 out), odd tiles a Schraudolph exp on the DVE (one
tensor_scalar building fp8e4m3 bit patterns in int8; its +-4% equi-
ripple error is common-mode across the softmax and mostly cancels).
Softmax and LN reciprocals run on GPSIMD as a Mitchell bit-trick seed +
one Newton step (standard convert/mult ops; GPSIMD cannot touch PSUM,
so Act first copies the denominator row to SBUF). The seed produces
-1/d; the broadcast matmul's stationary is -1 to flip it back.

Engines have in-order queues, so issue order is the main scheduling
tool: K/V projections are issued inside the LN0 scalar chain, each
attention iteration's epilogue is deferred into the next iteration's
key loop, and LN1/FFN/output-DMA are chunked per 512 tokens and
interleaved into the tail attention iterations.
"""

import contextlib

import numpy as np

B, N, D = 4, 2048, 256
H, DH, DFF = 8, 32, 512
P = 128
NTOK = N // 2            # query tokens per core
NQT = NTOK // 512        # q tiles of 512
NKT = N // P             # key tiles of 128
SCALE = 1.0 / 16.0
EPS = 1e-5
DSLOT = 512              # padded feature-slot space for Q/O

# Schraudolph exp constants for bf16 bit patterns in int16:
#   bits = round(s * SCALE*128*log2(e) + (127*128 - c8))
LOG2E = 1.4426950408889634
SCH_A = SCALE * 128.0 * LOG2E
SCH_B = 127.0 * 128.0 - 366392.5 / 65536.0
# Mitchell reciprocal seed: bits(y0) = C2 - bits(d); one Newton step
# (u-2)*y0 then gives -1/d to ~0.3%.
RCP_C2 = 2129834424.0

_NC_CACHE = {}


def _slot(h, i):
    return (h // 2) * P + 64 * (h % 2) + i


def _build_nc():
    import concourse.mybir as mybir
    import concourse.tile as tile
    from concourse import bacc

    f32 = mybir.dt.float32
    bf16 = mybir.dt.bfloat16
    fp8 = mybir.dt.float8e4
    i16 = mybir.dt.int16
    i32 = mybir.dt.int32
    AF = mybir.ActivationFunctionType
    ALU = mybir.AluOpType
    DR = mybir.MatmulPerfMode.DoubleRow

    nc = bacc.Bacc("TRN2", target_bir_lowering=False, debug=False)

    xt_d = nc.dram_tensor("xt", [D, NTOK], bf16, kind="ExternalInput")
    yt_d = nc.dram_tensor("yt", [D, N], bf16, kind="ExternalInput")
    wq_d = nc.dram_tensor("wq", [D, DSLOT], bf16, kind="ExternalInput")
    bq_d = nc.dram_tensor("bq", [DSLOT], f32, kind="ExternalInput")
    wk_d = nc.dram_tensor("wk", [D, DSLOT], bf16, kind="ExternalInput")
    bk_d = nc.dram_tensor("bk", [DSLOT], f32, kind="ExternalInput")
    wv_d = nc.dram_tensor("wv", [D, H * 33], bf16, kind="ExternalInput")
    w1_d = nc.dram_tensor("w1", [DSLOT, DFF], bf16, kind="ExternalInput")
    b1_d = nc.dram_tensor("b1", [DFF], f32, kind="ExternalInput")
    w2_d = nc.dram_tensor("w2", [DFF, DSLOT], bf16, kind="ExternalInput")
    b2_d = nc.dram_tensor("b2", [DSLOT], f32, kind="ExternalInput")
    out_d = nc.dram_tensor("out_t", [D, NTOK], f32, kind="ExternalOutput")

    with tile.TileContext(nc) as tc, contextlib.ExitStack() as ctx:
        const = ctx.enter_context(tc.tile_pool(name="const", bufs=1))
        big = ctx.enter_context(tc.tile_pool(name="big", bufs=1))
        scratch = ctx.enter_context(tc.tile_pool(name="scratch", bufs=1))
        apool = ctx.enter_context(tc.tile_pool(name="apool", bufs=4))
        # PSUM: scores 2x[128,1024]=4 banks, av 2, bc 1, proj 1.
        scores_pool = ctx.enter_context(
            tc.tile_pool(name="scoresp", bufs=2, space="PSUM"))
        av_pool = ctx.enter_context(tc.tile_pool(name="avp", bufs=2, space="PSUM"))
        bc_pool = ctx.enter_context(tc.tile_pool(name="bcp", bufs=1, space="PSUM"))
        proj_pool = ctx.enter_context(tc.tile_pool(name="projp", bufs=1, space="PSUM"))

        # ---- constants / inputs (K-proj inputs first) ---------------------
        yt_s = big.tile([P, 2, N], bf16)
        nc.sync.dma_start(yt_s[:], yt_d.rearrange("(o p) t -> p o t", p=P))
        wk_s = const.tile([P, 2, DSLOT], bf16)
        nc.sync.dma_start(wk_s[:], wk_d.rearrange("(o p) m -> p o m", p=P))
        xt_s = big.tile([P, 2, NTOK], bf16)
        nc.sync.dma_start(xt_s[:], xt_d.rearrange("(o p) t -> p o t", p=P))
        wv_s = const.tile([P, 2, H * 33], bf16)
        nc.sync.dma_start(wv_s[:], wv_d.rearrange("(o p) m -> p o m", p=P))
        wq_s = const.tile([P, 2, DSLOT], bf16)
        nc.sync.dma_start(wq_s[:], wq_d.rearrange("(o p) m -> p o m", p=P))
        w1_s = const.tile([P, 4, DFF], bf16)
        nc.sync.dma_start(w1_s[:], w1_d.rearrange("(o p) m -> p o m", p=P))
        w2_s = const.tile([P, 4, DSLOT], bf16)
        nc.sync.dma_start(w2_s[:], w2_d.rearrange("(o p) m -> p o m", p=P))
        bq_s = const.tile([P, 4], f32)
        nc.sync.dma_start(bq_s[:], bq_d.rearrange("(m p) -> p m", p=P))
        bk_s = const.tile([P, 4], f32)
        nc.sync.dma_start(bk_s[:], bk_d.rearrange("(m p) -> p m", p=P))
        b1_s = const.tile([P, 4], f32)
        nc.sync.dma_start(b1_s[:], b1_d.rearrange("(m p) -> p m", p=P))
        b2_s = const.tile([P, 4], f32)
        nc.sync.dma_start(b2_s[:], b2_d.rearrange("(m p) -> p m", p=P))

        ones_s = const.tile([P, 512], bf16)
        nc.vector.memset(ones_s[:], 1.0)
        negs_s = const.tile([P, 512], bf16)
        nc.vector.memset(negs_s[:], -1.0)
        eps_s = const.tile([1, 1], f32)
        nc.vector.memset(eps_s[:], EPS)


        # ---- shared LN scratch --------------------------------------------
        mean = scratch.tile([1, NTOK], f32, tag="mean")
        mean_b = scratch.tile([1, NTOK], bf16, tag="mean_b")
        rstd_b = scratch.tile([1, NTOK], bf16, tag="rstd_b")
        lt = scratch.tile([1, NTOK], f32, tag="lntmp")
        m2 = scratch.tile([1, NTOK], f32, tag="m2")
        mb_sb = scratch.tile([P, NTOK], bf16, tag="mb_sb")
        rb_sb = scratch.tile([P, NTOK], bf16, tag="rb_sb")
        rsf = scratch.tile([1, NTOK], f32, tag="rsf")

        def ln_sums(src, sq, no, hf):
            """Square already computed into sq; accumulate chunk sums into one
            proj_pool tile (sx at row 0, sq at row 32) -> mean / E[x^2]."""
            cs = slice(hf * 512, hf * 512 + 512)
            ps = proj_pool.tile([P, 512], f32, tag="proj", name="lnsum")
            for o in range(no):
                nc.tensor.matmul(ps[0:1, :], lhsT=ones_s[:, 0:1],
                                 rhs=src[:, o, cs],
                                 start=(o == 0), stop=(o == no - 1),
                                 tile_position=(0, 0), skip_group_check=True)
                nc.tensor.matmul(ps[32:33, :], lhsT=ones_s[:, 0:1],
                                 rhs=sq[:, o, cs],
                                 start=(o == 0), stop=(o == no - 1),
                                 tile_position=(0, 32), skip_group_check=True)
            nc.vector.tensor_scalar_mul(mean[0:1, cs], ps[0:1, :], 1.0 / D)
            nc.vector.tensor_scalar_mul(lt[0:1, cs], ps[32:33, :], 1.0 / D)

        def ln_finish(src, dst, no, hf, norm_eng=None):
            """rstd for chunk hf, broadcast, normalize src->dst. norm_eng
            picks the engine for the normalize multiplies: GPSIMD only when
            the chain is hidden under attention (its ~2us/op overhead is
            fatal on the phase-A / tail critical paths, where DVE wins)."""
            cs = slice(hf * 512, hf * 512 + 512)
            nc.vector.tensor_tensor(out=m2[0:1, cs], in0=mean[0:1, cs],
                                    in1=mean[0:1, cs], op=ALU.mult)
            nc.vector.tensor_tensor(out=lt[0:1, cs], in0=lt[0:1, cs],
                                    in1=m2[0:1, cs], op=ALU.subtract)
            nc.scalar.activation(out=lt[0:1, cs], in_=lt[0:1, cs], func=AF.Sqrt,
                                 bias=eps_s[:])
            nc.vector.reciprocal(out=rsf[0:1, cs], in_=lt[0:1, cs])
            nc.vector.tensor_copy(out=rstd_b[0:1, cs], in_=rsf[0:1, cs])
            nc.vector.tensor_copy(out=mean_b[0:1, cs], in_=mean[0:1, cs])
            br = scores_pool.tile([P, 1024], f32, tag="scores", name="br")
            nc.tensor.matmul(br[:, 0:512], lhsT=ones_s[0:1, 0:P],
                             rhs=mean_b[0:1, cs], start=True, stop=True)
            nc.tensor.matmul(br[:, 512:1024], lhsT=ones_s[0:1, 0:P],
                             rhs=rstd_b[0:1, cs], start=True, stop=True)
            nc.scalar.activation(out=mb_sb[:, cs], in_=br[:, 0:512], func=AF.Copy)
            nc.scalar.activation(out=rb_sb[:, cs], in_=br[:, 512:1024],
                                 func=AF.Copy)
            eng = norm_eng if norm_eng is not None else nc.vector
            for o in range(no):
                eng.tensor_tensor(out=dst[:, o, cs], in0=src[:, o, cs],
                                  in1=mb_sb[:, cs], op=ALU.subtract)
                eng.tensor_tensor(out=dst[:, o, cs], in0=dst[:, o, cs],
                                  in1=rb_sb[:, cs], op=ALU.mult)

        # ---- phase A: LN0 (sums), K/V proj, LN0 finish, Q proj -------------
        xln_s = big.tile([P, 2, NTOK], bf16)
        oln_s = big.tile([P, 4, NTOK], bf16)
        sq0 = oln_s[:, 0:2, :]                 # borrow as Square scratch
        nc.scalar.activation(out=sq0[:], in_=xt_s[:], func=AF.Square)
        for hf in range(NQT):
            ln_sums(xt_s, sq0, 2, hf)

        # K proj straight to fp8 (scores are the only K consumer). The LN0
        # finish chain is issued after two K columns: its broadcast matmuls
        # then sit mid-queue on the PE and their DVE/act inputs are ready by
        # the time the PE drains the remaining K/V work ahead of them.
        kt8_pre = big.tile([P, 4, N], fp8)
        for mt in range(4):
            if mt == 2:
                for hf in range(NQT):
                    ln_finish(xt_s, xln_s, 2, hf)
            for nt in range(N // 512):
                ns_ = slice(nt * 512, nt * 512 + 512)
                ps = proj_pool.tile([P, 512], f32, tag="proj", name="ps")
                for o in range(2):
                    nc.tensor.matmul(ps[:], lhsT=wk_s[:, o, mt * P:mt * P + P],
                                     rhs=yt_s[:, o, ns_],
                                     start=(o == 0), stop=(o == 1))
                if nt % 2 == 0:
                    nc.scalar.activation(out=kt8_pre[:, mt, ns_], in_=ps[:],
                                         func=AF.Identity,
                                         bias=bk_s[:, mt:mt + 1])
                else:
                    nc.vector.tensor_scalar_add(kt8_pre[:, mt, ns_], ps[:],
                                                bk_s[:, mt:mt + 1])
        # relayout to 16-partition dh-subtile strips for DoubleRow scores
        kt8 = big.tile([P, 2, 2, N], fp8)
        for h in range(H):
            b, l = 32 * (h % 4), h // 4
            for s in range(2):
                nc.sync.dma_start(
                    kt8[b:b + 16, l, s, :],
                    kt8_pre[64 * (h % 2) + 16 * s:64 * (h % 2) + 16 * s + 16,
                            h // 2, :])
        # V in natural [token, dout] layout, 33-wide head blocks ([Vh | ones])
        v_s = big.tile([P, NKT, H * 33], bf16)
        for tt in range(NKT):
            ts_ = slice(tt * P, tt * P + P)
            ps = proj_pool.tile([P, 512], f32, tag="proj", name="ps")[:, 0:H * 33]
            for o in range(2):
                nc.tensor.matmul(ps[:], lhsT=yt_s[:, o, ts_],
                                 rhs=wv_s[:, o, :], start=(o == 0), stop=(o == 1))
            if tt % 2 == 0:
                nc.scalar.activation(out=v_s[:, tt, :], in_=ps[:], func=AF.Copy)
            else:
                nc.vector.tensor_copy(out=v_s[:, tt, :], in_=ps[:])
        for h in range(H):
            nc.gpsimd.memset(v_s[:, :, 33 * h + 32], 1.0)

        qt_s = big.tile([P, 4, NTOK], bf16)
        qt8_pre = big.tile([P, 4, NTOK], fp8)
        for mt in range(4):
            for nt in range(NQT):
                ns_ = slice(nt * 512, nt * 512 + 512)
                ps = proj_pool.tile([P, 512], f32, tag="proj", name="ps")
                for o in range(2):
                    nc.tensor.matmul(ps[:], lhsT=wq_s[:, o, mt * P:mt * P + P],
                                     rhs=xln_s[:, o, ns_],
                                     start=(o == 0), stop=(o == 1))
                nc.scalar.activation(out=qt_s[:, mt, ns_], in_=ps[:],
                                     func=AF.Identity, bias=bq_s[:, mt:mt + 1])
                nc.vector.tensor_scalar_add(qt8_pre[:, mt, ns_], ps[:],
                                            bq_s[:, mt:mt + 1])
        qt8 = big.tile([P, 2, 2, NTOK], fp8)
        for h in range(H):
            b, l = 32 * (h % 4), h // 4
            for s in range(2):
                nc.sync.dma_start(
                    qt8[b:b + 16, l, s, :],
                    qt8_pre[64 * (h % 2) + 16 * s:64 * (h % 2) + 16 * s + 16,
                            h // 2, :])

        # ---- phase B (attention) with phase C (LN1+FFN) interleaved --------
        ot_s = big.tile([P, 4, NTOK], bf16)
        nc.gpsimd.memset(ot_s[32:64, :, :], 0.0)
        nc.gpsimd.memset(ot_s[96:128, :, :], 0.0)
        h_s = big.tile([P, 4, NTOK], bf16)
        outt_s = big.tile([P, 4, NTOK], f32)
        rcf_s = scratch.tile([P, 512], f32, tag="rcf")
        rc_s = scratch.tile([P, 512], bf16, tag="rc")

        def make_epilogue(pr, qt, av):
            qs_ = slice(qt * 512, qt * 512 + 512)

            def emit():
                for jj in range(2):
                    st = 64 * jj
                    nc.vector.reciprocal(out=rcf_s[st + 32:st + 33, :],
                                         in_=av[st + 32:st + 33, :])
                    nc.vector.tensor_copy(out=rc_s[st + 32:st + 33, :],
                                          in_=rcf_s[st + 32:st + 33, :])
                bc = bc_pool.tile([P, 512], f32, tag="bc", name="bc")
                for jj in range(2):
                    st = 64 * jj
                    nc.tensor.matmul(bc[st:st + 32, :],
                                     lhsT=ones_s[st + 32:st + 33, 0:32],
                                     rhs=rc_s[st + 32:st + 33, :],
                                     start=True, stop=True,
                                     tile_position=(st + 32, st))
                avs = scratch.tile([P, 512], f32, tag="avs", name="avs")
                nrm = scratch.tile([P, 512], bf16, tag="nrm", name="nrm")
                for jj in range(2):
                    st = 64 * jj
                    nc.scalar.activation(out=avs[st:st + 32, :],
                                         in_=av[st:st + 32, :], func=AF.Copy)
                    nc.vector.tensor_tensor(out=nrm[st:st + 32, :],
                                            in0=avs[st:st + 32, :],
                                            in1=bc[st:st + 32, :],
                                            op=ALU.mult)
                    nc.gpsimd.tensor_tensor(out=ot_s[st:st + 32, pr, qs_],
                                            in0=nrm[st:st + 32, :],
                                            in1=qt_s[st:st + 32, pr, qs_],
                                            op=ALU.add)
            return emit

        def ffn1_chunk(hf):
            cs = slice(hf * 512, hf * 512 + 512)
            for mt in range(DFF // P):
                ms = slice(mt * P, mt * P + P)
                ps = proj_pool.tile([P, 512], f32, tag="proj", name="ps")
                for o in range(4):
                    nc.tensor.matmul(ps[:], lhsT=w1_s[:, o, ms],
                                     rhs=oln_s[:, o, cs],
                                     start=(o == 0), stop=(o == 3))
                nc.scalar.activation(out=h_s[:, mt, cs], in_=ps[:],
                                     func=AF.Gelu, bias=b1_s[:, mt:mt + 1])

        def ffn2_chunk(hf):
            cs = slice(hf * 512, hf * 512 + 512)
            for mt in range(4):
                ms = slice(mt * P, mt * P + P)
                ps = proj_pool.tile([P, 512], f32, tag="proj", name="ps")
                for o in range(4):
                    nc.tensor.matmul(ps[:], lhsT=w2_s[:, o, ms],
                                     rhs=h_s[:, o, cs],
                                     start=(o == 0), stop=(o == 3))
                nc.vector.scalar_tensor_tensor(
                    out=outt_s[:, mt, cs], in0=ps[:],
                    scalar=b2_s[:, mt:mt + 1], in1=ot_s[:, mt, cs],
                    op0=ALU.add, op1=ALU.add)
            for h in range(H):
                nc.sync.dma_start(
                    out_d[32 * h:32 * h + 32, cs],
                    outt_s[64 * (h % 2):64 * (h % 2) + 32, h // 2, cs])

        def ln1_square_sums(hf):
            nc.scalar.activation(out=h_s[:, :, hf * 512:hf * 512 + 512],
                                 in_=ot_s[:, :, hf * 512:hf * 512 + 512],
                                 func=AF.Square)
            ln_sums(ot_s, h_s, 4, hf)

        pending = None       # previous iteration's epilogue
        deferred = []        # chunked LN1/FFN stages

        for idx, (qt, pr) in enumerate(
                [(q, p) for q in range(NQT) for p in range(4)]):
            qs_ = slice(qt * 512, qt * 512 + 512)
            av = av_pool.tile([P, 512], f32, tag="av", name="av")

            def av_mm(kt, a):
                # AV for key tile kt, issued one kt late so it never blocks
                # the next score matmuls at the head of the in-order PE queue
                for jj in range(2):
                    h = 2 * pr + jj
                    st = 64 * jj
                    nc.tensor.matmul(
                        av[st:st + 33, :],
                        lhsT=v_s[:, kt, 33 * h:33 * h + 33],
                        rhs=a[:, jj * 512:jj * 512 + 512],
                        start=(kt == 0), stop=(kt == NKT - 1),
                        tile_position=(0, st),
                        skip_group_check=True)

            prev_av = None
            for kt in range(NKT):
                ks_ = slice(kt * P, kt * P + P)
                sp = scores_pool.tile([P, 1024], f32, tag="scores", name="sp")
                for jj in range(2):
                    h = 2 * pr + jj
                    b, l = 32 * (h % 4), h // 4
                    nc.tensor.matmul(
                        sp[:, jj * 512:jj * 512 + 512],
                        lhsT=kt8[b:b + 16, l, :, ks_],
                        rhs=qt8[b:b + 16, l, :, qs_],
                        start=True, stop=True, perf_mode=DR,
                        tile_position=(b, 0))
                if kt % 4 != 1:
                    ab = apool.tile([P, 1024], bf16, tag="a", name="a")
                    nc.scalar.activation(out=ab[:], in_=sp[:], func=AF.Exp,
                                         scale=SCALE)
                    a = ab[:]
                else:
                    ai = apool.tile([P, 1024], i16, tag="a", name="a")
                    nc.vector.tensor_scalar(
                        out=ai[:], in0=sp[:], scalar1=SCH_A, scalar2=SCH_B,
                        op0=ALU.mult, op1=ALU.add)
                    a = ai[:].bitcast(bf16)
                if prev_av is not None:
                    av_mm(kt - 1, prev_av)
                prev_av = a
                if kt == 3 and pending is not None:
                    pending()
                    pending = None
                elif kt in (8, 12) and deferred:
                    deferred.pop(0)()
            av_mm(NKT - 1, prev_av)
            pending = make_epilogue(pr, qt, av)
            if idx == 3:
                deferred.extend([
                    lambda: ln1_square_sums(0),
                    lambda: ln_finish(ot_s, oln_s, 4, 0, norm_eng=nc.gpsimd),
                    lambda: ffn1_chunk(0),
                    lambda: ffn2_chunk(0),
                ])
        pending()
        ln1_square_sums(1)
        ln_finish(ot_s, oln_s, 4, 1)
        ffn1_chunk(1)
        ffn2_chunk(1)

    nc.compile()
    return nc


def get_nc():
    if "nc" not in _NC_CACHE:
        _NC_CACHE["nc"] = _build_nc()
    return _NC_CACHE["nc"]


def _host_prep(inputs):
    import ml_dtypes

    bf = ml_dtypes.bfloat16
    f = lambda k: np.asarray(inputs[k], np.float32)
    x, y = f("x"), f("y")
    Wq, bq, Wk, bk, Wv, bv = f("Wq"), f("bq"), f("Wk"), f("bk"), f("Wv"), f("bv")
    W1, b1, W2, b2 = f("W1"), f("b1"), f("W2"), f("b2")
    ln0_g, ln0_b, ln1_g, ln1_b = f("ln0_g"), f("ln0_b"), f("ln1_g"), f("ln1_b")
    # fold LN affines into the following linears; fold bv into bq (sum(A)=1)
    Wq_eff = Wq * ln0_g[None, :]
    bq_eff = bq + Wq @ ln0_b + bv
    W1_eff = W1 * ln1_g[None, :]
    b1_eff = b1 + W1 @ ln1_b

    # permutation: original feature d=32h+i -> slot(h,i) in the 512 space
    slots = np.zeros(D, np.int64)
    for h in range(H):
        for i in range(DH):
            slots[DH * h + i] = _slot(h, i)

    wq_h = np.zeros((D, DSLOT), np.float32)
    wq_h[:, slots] = Wq_eff.T            # [din, dout-slot]
    bq_h = np.zeros(DSLOT, np.float32)
    bq_h[slots] = bq_eff
    wk_h = np.zeros((D, DSLOT), np.float32)
    wk_h[:, slots] = Wk.T
    bk_h = np.zeros(DSLOT, np.float32)
    bk_h[slots] = bk
    wv_h = np.zeros((D, H * 33), np.float32)
    for h in range(H):
        wv_h[:, 33 * h:33 * h + 32] = Wv.T[:, DH * h:DH * h + DH]
    w1_h = np.zeros((DSLOT, DFF), np.float32)
    w1_h[slots, :] = W1_eff.T            # [din-slot, dff]
    w2_h = np.zeros((DFF, DSLOT), np.float32)
    w2_h[:, slots] = W2.T
    b2_h = np.zeros(DSLOT, np.float32)
    b2_h[slots] = b2

    wq_h = wq_h.astype(bf)
    wk_h = wk_h.astype(bf)
    wv_h = wv_h.astype(bf)
    w1_h = w1_h.astype(bf)
    w2_h = w2_h.astype(bf)

    in_maps = []
    for core in range(8):
        b, half = core // 2, core % 2
        in_maps.append({
            "xt": np.ascontiguousarray(
                x[b, half * NTOK:(half + 1) * NTOK, :].T).astype(bf),
            "yt": np.ascontiguousarray(y[b].T).astype(bf),
            "wq": wq_h, "bq": bq_h, "wk": wk_h, "bk": bk_h, "wv": wv_h,
            "w1": w1_h, "b1": np.ascontiguousarray(b1_eff), "w2": w2_h,
            "b2": b2_h,
        })
    return in_maps


def kernel_with_results(inputs, **run_kwargs):
    from concourse.bass_utils import run_bass_kernel_spmd
    nc = get_nc()
    in_maps = _host_prep(inputs)
    res = run_bass_kernel_spmd(nc, in_maps, core_ids=list(range(8)), **run_kwargs)
    out = np.empty((B, N, D), np.float32)
    for core in range(8):
        b, half = core // 2, core % 2
        out[b, half * NTOK:(half + 1) * NTOK, :] = res.results[core]["out_t"].T
    return out, res


def kernel(**inputs):
    out, _ = kernel_with_results(inputs)
    return out
